# revision 1
# baseline (speedup 1.0000x reference)
"""CBAM kernel for Trainium2, 8-way batch-parallel SPMD.

Computes out = x^2 * (att_c[b,c] + sigmoid(conv(spatial_stats))[b,l]) where
att_c = sigmoid(mlp(mean_L x) + mlp(max_L x)), matching the CBAM reference.

Layout per core: 4 batches; each batch x[4096, 256] lives in SBUF as one
[128, 8192] tensor (partition = l % 128, free column = 256*(l//128) + c).
Engine split per batch:
  PE   : channel-sum (ones-matmul accumulation), transposes, MLP, conv
         (conv over L is a banded-Toeplitz matmul with host-built weights)
  ACT  : spatial sums (copy w/ accum_out), squares, sigmoids, relu
  DVE  : spatial max (one 3D reduce), max-tree folds, final fused
         (att + sig) * x^2 via scalar_tensor_tensor
  POOL : first max-tree fold
"""

import numpy as np
from contextlib import ExitStack

import concourse.bacc as bacc
import concourse.bass as bass
import concourse.tile as tile
import concourse.mybir as mybir
from concourse.bass_utils import run_bass_kernel_spmd

AF = mybir.ActivationFunctionType
ALU = mybir.AluOpType
AX = mybir.AxisListType
FP32 = mybir.dt.float32

N_CORES = 8
B_FULL = 32
NB = B_FULL // N_CORES  # batches per core = 4
L = 4096
C = 256
HID = 16
P = 128
NT = L // P  # 32 L-tiles per batch
SQW = 2048   # ACT square slice width (8 tiles)

_CACHE: dict = {}


def _build_body(ctx: ExitStack, tc, out_d, x_d, w1_d, b1_d, w2b_d, cm_d, cc_d,
                ones_d, id_d, rc_d, reps=1):
    nc = tc.nc

    const = ctx.enter_context(tc.tile_pool(name="const", bufs=1))
    xpool = ctx.enter_context(tc.tile_pool(name="x", bufs=2))
    mpool = ctx.enter_context(tc.tile_pool(name="maxtree", bufs=2))
    spool = ctx.enter_context(tc.tile_pool(name="stats", bufs=2))
    sqpool = ctx.enter_context(tc.tile_pool(name="sq", bufs=3))
    opool = ctx.enter_context(tc.tile_pool(name="outt", bufs=8))
    dpool = ctx.enter_context(tc.tile_pool(name="dummy", bufs=2))
    apool = ctx.enter_context(tc.tile_pool(name="att", bufs=2))
    pacc = ctx.enter_context(tc.tile_pool(name="pacc", bufs=2, space="PSUM"))
    pwork = ctx.enter_context(tc.tile_pool(name="pwork", bufs=4, space="PSUM"))

    w1 = const.tile([P, 2 * (HID + 1)], FP32)
    nc.sync.dma_start(w1[:], w1_d[:])
    b1 = const.tile([HID + 1, 1], FP32)
    nc.sync.dma_start(b1[:], b1_d[:])
    w2b = const.tile([HID + 1, C], FP32)
    nc.sync.dma_start(w2b[:], w2b_d[:])
    cmain = const.tile([P, 2 * P], FP32)
    nc.sync.dma_start(cmain[:], cm_d[:])
    ccorn = const.tile([P, 4 * P], FP32)
    nc.sync.dma_start(ccorn[:], cc_d[:])
    ones = const.tile([P, P], FP32)
    nc.sync.dma_start(ones[:], ones_d[:])
    ident = const.tile([P, P], FP32)
    nc.sync.dma_start(ident[:], id_d[:])
    redcol = const.tile([P, 1], FP32)
    nc.sync.dma_start(redcol[:], rc_d[:])

    HN = NT // 2  # 16 tiles per half-batch
    for b in [b for _ in range(reps) for b in range(NB)]:
        # x lives in two half-batch tensors so compute starts once the
        # first half has landed.
        xh = [xpool.tile([P, HN * C], FP32, tag=f"xb{h}", name=f"xb{h}")
              for h in range(2)]
        for t in range(NT):
            nc.sync.dma_start(xh[t // HN][:, C * (t % HN):C * (t % HN + 1)],
                              x_d[b, P * t:P * (t + 1), :])

        def xtile(t):
            return xh[t // HN][:, C * (t % HN):C * (t % HN + 1)]

        # ---- channel mean over L (PE): lhsT = x tile half, rhs = 1/L col;
        # psum [128, 1] per channel-half accumulates in channel-major ----
        # lhsT = 1/L column (stationary, loaded once), x streams as rhs:
        # pcs[0, c] accumulates mean over L.
        pcs = pacc.tile([1, C], FP32, tag="pcs")
        for t in range(NT):
            nc.tensor.matmul(pcs[:], redcol[:], xtile(t)[:],
                             start=(t == 0), stop=(t == NT - 1),
                             skip_group_check=True)

        # ---- spatial sum over C: ACT copies with accum_out ----
        sum_s = spool.tile([P, NT], FP32, tag="sum_s")
        for t in range(NT):
            dummy = dpool.tile([P, C], FP32, tag="dummy")
            nc.scalar.activation(dummy[:], xtile(t)[:],
                                 AF.Identity, accum_out=sum_s[:, t:t + 1])

        # ---- spatial max over C: one 3D reduce (DVE) per half ----
        max_s = spool.tile([P, NT], FP32, tag="max_s")
        for h in range(2):
            nc.vector.tensor_reduce(
                max_s[:, HN * h:HN * (h + 1)],
                xh[h][:].rearrange("p (t c) -> p t c", c=C),
                axis=AX.X, op=ALU.max)

        # ---- channel max over L: fold tree then transpose+reduce ----
        half = HN * C  # 4096
        mb = mpool.tile([P, half], FP32, tag="mb")
        nc.vector.tensor_max(mb[:], xh[0][:], xh[1][:])
        w = half // 2
        while w >= C:
            nc.vector.tensor_max(mb[:, 0:w], mb[:, 0:w], mb[:, w:2 * w])
            w //= 2

        stats_cm = spool.tile([P, 4], FP32, tag="stats_cm")
        avg_row = spool.tile([1, C], FP32, tag="avg_row")
        nc.scalar.activation(avg_row[:], pcs[:], AF.Copy)
        # chan-max without PE transposes: 32x32 block transpose (DVE) puts
        # channels along free within blocks; reduce in-block, then fold the
        # four partition quadrants.
        bt = spool.tile([P, C], FP32, tag="bt")
        nc.vector.transpose(bt[:], mb[:, 0:C])
        red = spool.tile([P, 8], FP32, tag="red")
        nc.vector.tensor_reduce(red[:],
                                bt[:].rearrange("p (bj s) -> p bj s", s=32),
                                axis=AX.X, op=ALU.max)
        # DVE ops need equal base partitions, so fold the four partition
        # quadrants by gathering them into columns with tiny DMAs first.
        cm32 = spool.tile([32, 32], FP32, tag="cm32")
        for a in range(4):
            nc.gpsimd.dma_start(cm32[:, 8 * a:8 * (a + 1)],
                                red[32 * a:32 * (a + 1), :])
        cmf = spool.tile([32, 8], FP32, tag="cmf")
        nc.vector.tensor_reduce(cmf[:],
                                cm32[:].rearrange("r (a bj) -> r bj a", a=4),
                                axis=AX.X, op=ALU.max)
        # scatter into channel-major stats via tiny stream-matched DMAs on
        # the idle gpsimd queue: cmf[r, bj] is the max of channel 32*bj+r.
        for h in range(2):
            nc.gpsimd.dma_start(stats_cm[:, 2 * h:2 * h + 1],
                                avg_row[0:1, P * h:P * (h + 1)])
        for bj in range(8):
            q = 32 * (bj % 4)
            nc.gpsimd.dma_start(stats_cm[q:q + 32, 2 * (bj // 4) + 1:
                                         2 * (bj // 4) + 2],
                                cmf[:, bj:bj + 1])

        # ---- shared MLP: att logits broadcast over partitions via matmul ----
        # Row HID (=16) carries a constant: lhsT col 16 is zero, relu bias row
        # 16 is 1.0, so hsb[16, :] = 1, h2[16] = 2 — which multiplies the b2
        # row of w2b to add the 2*b2 term.
        ph = pwork.tile([HID + 1, 2], FP32, tag="pwork")
        nc.tensor.matmul(ph[:], w1[:, 0:HID + 1], stats_cm[:, 0:2],
                         start=True, stop=False, skip_group_check=True)
        nc.tensor.matmul(ph[:], w1[:, HID + 1:2 * (HID + 1)], stats_cm[:, 2:4],
                         start=False, stop=True, skip_group_check=True)
        hsb = spool.tile([HID + 1, 2], FP32, tag="hsb")
        nc.scalar.activation(hsb[:], ph[:], AF.Relu, bias=b1[:])
        h2 = spool.tile([HID + 1, 1], FP32, tag="h2")
        nc.vector.tensor_add(h2[:], hsb[:, 0:1], hsb[:, 1:2])
        h2r = spool.tile([HID + 1, P], FP32, tag="h2r")
        nc.vector.tensor_scalar_mul(h2r[:], ones[0:HID + 1, :], h2[:])
        po = pwork.tile([P, C], FP32, tag="pwork")
        nc.tensor.matmul(po[:], h2r[:], w2b[:], start=True, stop=True,
                         skip_group_check=True)
        att = apool.tile([P, C], FP32, tag="att")
        nc.scalar.activation(att[:], po[:], AF.Sigmoid)

        # ---- spatial conv over L: banded-Toeplitz matmuls ----
        pc = pwork.tile([P, NT], FP32, tag="pwork")
        nc.tensor.matmul(pc[:, :], cmain[:, 0:P], sum_s[:],
                         start=True, stop=False, skip_group_check=True)
        nc.tensor.matmul(pc[:, :], cmain[:, P:2 * P], max_s[:],
                         start=False, stop=False, skip_group_check=True)
        nc.tensor.matmul(pc[:, 1:NT], ccorn[:, 0:P], sum_s[:, 0:NT - 1],
                         start=False, stop=False, skip_group_check=True)
        nc.tensor.matmul(pc[:, 1:NT], ccorn[:, P:2 * P],
                         max_s[:, 0:NT - 1],
                         start=False, stop=False, skip_group_check=True)
        nc.tensor.matmul(pc[:, 0:NT - 1], ccorn[0:3, 2 * P:3 * P],
                         sum_s[0:3, 1:NT],
                         start=False, stop=False, skip_group_check=True)
        nc.tensor.matmul(pc[:, 0:NT - 1], ccorn[0:3, 3 * P:4 * P],
                         max_s[0:3, 1:NT],
                         start=False, stop=True, skip_group_check=True)
        sig = spool.tile([P, NT], FP32, tag="sig")
        nc.scalar.activation(sig[:], pc[:], AF.Sigmoid)

        # ---- final: out = (att + sig) * x^2 ----
        sph = HN * C // SQW  # square slices per half
        sqs = []
        for s in range(NT * C // SQW):
            sq = sqpool.tile([P, SQW], FP32, tag="sq")
            nc.scalar.activation(
                sq[:], xh[s // sph][:, SQW * (s % sph):SQW * (s % sph + 1)],
                AF.Square)
            sqs.append(sq)
        tps = SQW // C  # tiles per square slice
        for t in range(NT):
            ot = opool.tile([P, C], FP32, tag="ot")
            sq = sqs[t // tps]
            off = C * (t % tps)
            nc.vector.scalar_tensor_tensor(ot[:], att[:], sig[:, t:t + 1],
                                           sq[:, off:off + C],
                                           op0=ALU.add, op1=ALU.mult)
            nc.sync.dma_start(out_d[b, P * t:P * (t + 1), :], ot[:])


def _build_nc(reps=1):
    nc = bacc.Bacc("TRN2", target_bir_lowering=False, debug=False,
                   enable_asserts=False, num_devices=N_CORES)
    x_d = nc.dram_tensor("xb", [NB, L, C], FP32, kind="ExternalInput").ap()
    w1_d = nc.dram_tensor("w1sb", [P, 2 * (HID + 1)], FP32, kind="ExternalInput").ap()
    b1_d = nc.dram_tensor("b1col", [HID + 1, 1], FP32, kind="ExternalInput").ap()
    w2b_d = nc.dram_tensor("w2b", [HID + 1, C], FP32, kind="ExternalInput").ap()
    cm_d = nc.dram_tensor("convmain", [P, 2 * P], FP32, kind="ExternalInput").ap()
    cc_d = nc.dram_tensor("convcorner", [P, 4 * P], FP32, kind="ExternalInput").ap()
    ones_d = nc.dram_tensor("ones", [P, P], FP32, kind="ExternalInput").ap()
    id_d = nc.dram_tensor("ident", [P, P], FP32, kind="ExternalInput").ap()
    rc_d = nc.dram_tensor("redcol", [P, 1], FP32, kind="ExternalInput").ap()
    out_d = nc.dram_tensor("out", [NB, L, C], FP32, kind="ExternalOutput").ap()

    with tile.TileContext(nc) as tc:
        with ExitStack() as ctx:
            _build_body(ctx, tc, out_d, x_d, w1_d, b1_d, w2b_d, cm_d, cc_d,
                        ones_d, id_d, rc_d, reps=reps)
    nc.compile()
    return nc


def get_nc(reps=1):
    key = f"nc{reps}"
    if key not in _CACHE:
        _CACHE[key] = _build_nc(reps=reps)
    return _CACHE[key]


def _prep_inputs(W1, b1, W2, b2, conv_w):
    """Host-side parameter preprocessing (shared across cores)."""
    W1 = np.asarray(W1, np.float32)
    W2 = np.asarray(W2, np.float32)
    b1 = np.asarray(b1, np.float32)
    b2 = np.asarray(b2, np.float32)
    conv_w = np.asarray(conv_w, np.float32)

    HB = HID + 1
    w1sb = np.zeros((P, 2 * HB), np.float32)
    for h in range(2):
        w1sb[:, HB * h:HB * h + HID] = W1[P * h:P * (h + 1), :]
    w2b = np.concatenate([W2, b2[None, :]], axis=0).astype(np.float32)
    b1col = np.concatenate([b1, [1.0]]).astype(np.float32).reshape(HB, 1)

    # Banded Toeplitz over two adjacent 128-blocks; avg band folds in the
    # 1/C spatial-mean scale (device computes raw channel sums).
    wa = conv_w[:, 0, 0] / C
    wm = conv_w[:, 1, 0]
    Wb_a = np.zeros((2 * P, 2 * P), np.float32)
    Wb_m = np.zeros((2 * P, 2 * P), np.float32)
    for i in range(2 * P):
        for k in range(7):
            j = i + k - 3
            if 0 <= j < 2 * P:
                Wb_a[i, j] = wa[k]
                Wb_m[i, j] = wm[k]
    cmain = np.concatenate([Wb_a[0:P, 0:P].T, Wb_m[0:P, 0:P].T], axis=1)
    # Corner lhsTs in one [128, 512] tensor. The prev-block ("lo") bands use
    # full K=128 (only rows 125-127 nonzero) so the rhs stays at base
    # partition 0 (PE requires base partition in {0, 32, 64}); the
    # next-block ("hi") bands are K=3 at rows 0-2.
    corn = np.zeros((P, 4 * P), np.float32)
    corn[:, 0:P] = Wb_a[P:2 * P, 0:P].T            # prev-block avg
    corn[:, P:2 * P] = Wb_m[P:2 * P, 0:P].T        # prev-block max
    corn[0:3, 2 * P:3 * P] = Wb_a[0:P, P:2 * P].T[0:3, :]   # next-block avg
    corn[0:3, 3 * P:4 * P] = Wb_m[0:P, P:2 * P].T[0:3, :]   # next-block max
    return {
        "w1sb": w1sb,
        "b1col": np.ascontiguousarray(b1col),
        "w2b": w2b,
        "convmain": np.ascontiguousarray(cmain),
        "convcorner": np.ascontiguousarray(corn),
        "ones": np.ones((P, P), np.float32),
        "ident": np.eye(P, dtype=np.float32),
        "redcol": np.full((P, 1), 1.0 / L, np.float32),
    }


def kernel(x, W1, b1, W2, b2, conv_w):
    nc = get_nc()
    x = np.asarray(x, np.float32)
    params = _prep_inputs(W1, b1, W2, b2, conv_w)
    in_maps = []
    for c in range(N_CORES):
        m = dict(params)
        m["xb"] = np.ascontiguousarray(x[NB * c:NB * (c + 1)])
        in_maps.append(m)
    _CACHE["last_in_maps"] = in_maps
    res = run_bass_kernel_spmd(nc, in_maps, list(range(N_CORES)))
    _CACHE["last_results"] = res
    return np.concatenate([res.results[c]["out"] for c in range(N_CORES)],
                          axis=0)


def _pjrt_exec(nc, in_maps, n_warm=2, n_time=8):
    """Build a sharded jit for nc, run it, return (best_wall_s, result)."""
    import time
    import jax
    import concourse.mybir as mybir_
    from concourse.bass2jax import (_bass_exec_p, install_neuronx_cc_hook,
                                    partition_id_tensor)
    from jax.experimental.shard_map import shard_map
    from jax.sharding import Mesh, PartitionSpec

    install_neuronx_cc_hook()
    partition_name = (nc.partition_id_tensor.name
                      if nc.partition_id_tensor else None)
    in_names, out_names, out_avals = [], [], []
    for alloc in nc.m.functions[0].allocations:
        if not isinstance(alloc, mybir_.MemoryLocationSet):
            continue
        name = alloc.memorylocations[0].name
        if alloc.kind == "ExternalInput":
            if name != partition_name:
                in_names.append(name)
        elif alloc.kind == "ExternalOutput":
            out_names.append(name)
            out_avals.append(jax.core.ShapedArray(
                tuple(alloc.tensor_shape), mybir_.dt.np(alloc.dtype)))
    n_params = len(in_names)
    all_in_names = list(in_names) + list(out_names)
    if partition_name is not None:
        all_in_names.append(partition_name)

    def _body(*args):
        operands = list(args)
        if partition_name is not None:
            operands.append(partition_id_tensor())
        return tuple(_bass_exec_p.bind(
            *operands,
            out_avals=tuple(out_avals),
            in_names=tuple(all_in_names),
            out_names=tuple(out_names),
            lowering_input_output_aliases=(),
            sim_require_finite=True,
            sim_require_nnan=True,
            nc=nc,
        ))

    devices = jax.devices()[:N_CORES]
    mesh = Mesh(np.asarray(devices), ("core",))
    nin = n_params + len(out_names)
    sharding = jax.sharding.NamedSharding(mesh, PartitionSpec("core"))
    fn = jax.jit(shard_map(
        _body, mesh=mesh,
        in_specs=(PartitionSpec("core"),) * nin,
        out_specs=(PartitionSpec("core"),) * len(out_names),
        check_rep=False))
    dev_args = [
        jax.device_put(np.concatenate(
            [np.asarray(in_maps[c][nm]) for c in range(N_CORES)], axis=0),
            sharding)
        for nm in in_names
    ]
    for av in out_avals:
        z = np.zeros((N_CORES * av.shape[0], *av.shape[1:]), av.dtype)
        dev_args.append(jax.device_put(z, sharding))

    for _ in range(n_warm):
        out = fn(*dev_args)
        jax.block_until_ready(out)
    best = float("inf")
    for _ in range(n_time):
        t0 = time.perf_counter()
        out = fn(*dev_args)
        jax.block_until_ready(out)
        best = min(best, time.perf_counter() - t0)
    result = np.asarray(out[0]).reshape(N_CORES * NB, L, C)
    return best, result


def bench_repeat(reps=8, n_time=10, in_maps=None):
    """Isolate device exec time: time a module doing the work `reps` times
    in-kernel vs once; slope = steady-state HW time per execution."""
    if in_maps is None:
        in_maps = _CACHE["last_in_maps"]
    t1, _ = _pjrt_exec(get_nc(1), in_maps, n_time=n_time)
    tr, result = _pjrt_exec(get_nc(reps), in_maps, n_time=n_time)
    per_exec_ns = (tr - t1) / (reps - 1) * 1e9
    return per_exec_ns, result, t1 * 1e9, tr * 1e9


def bench(n_iters=30, in_maps=None):
    """Time back-to-back NEFF executions with device-resident inputs.

    Mirrors bass2jax.run_bass_via_pjrt's multi-core path but without buffer
    donation so inputs (incl. zero-filled output buffers) stay reusable
    across iterations; reports amortized per-iteration wall time, which
    bounds true HW exec time from above by the per-dispatch overhead.
    """
    import time
    import jax
    import concourse.mybir as mybir_
    from concourse.bass2jax import (_bass_exec_p, install_neuronx_cc_hook,
                                    partition_id_tensor)
    from jax.experimental.shard_map import shard_map
    from jax.sharding import Mesh, PartitionSpec

    nc = get_nc()
    if in_maps is None:
        in_maps = _CACHE["last_in_maps"]
    install_neuronx_cc_hook()

    partition_name = (nc.partition_id_tensor.name
                      if nc.partition_id_tensor else None)
    in_names, out_names, out_avals, zero_outs = [], [], [], []
    for alloc in nc.m.functions[0].allocations:
        if not isinstance(alloc, mybir_.MemoryLocationSet):
            continue
        name = alloc.memorylocations[0].name
        if alloc.kind == "ExternalInput":
            if name != partition_name:
                in_names.append(name)
        elif alloc.kind == "ExternalOutput":
            shape = tuple(alloc.tensor_shape)
            dtype = mybir_.dt.np(alloc.dtype)
            out_names.append(name)
            out_avals.append(jax.core.ShapedArray(shape, dtype))
            zero_outs.append(np.zeros(shape, dtype))
    n_params = len(in_names)
    all_in_names = list(in_names) + list(out_names)
    if partition_name is not None:
        all_in_names.append(partition_name)

    def _body(*args):
        operands = list(args)
        if partition_name is not None:
            operands.append(partition_id_tensor())
        return tuple(_bass_exec_p.bind(
            *operands,
            out_avals=tuple(out_avals),
            in_names=tuple(all_in_names),
            out_names=tuple(out_names),
            lowering_input_output_aliases=(),
            sim_require_finite=True,
            sim_require_nnan=True,
            nc=nc,
        ))

    devices = jax.devices()[:N_CORES]
    mesh = Mesh(np.asarray(devices), ("core",))
    nin = n_params + len(out_names)
    sharded = jax.jit(shard_map(
        _body, mesh=mesh,
        in_specs=(PartitionSpec("core"),) * nin,
        out_specs=(PartitionSpec("core"),) * len(out_names),
        check_rep=False))

    concat_in = [
        np.concatenate([np.asarray(in_maps[c][nm]) for c in range(N_CORES)],
                       axis=0)
        for nm in in_names
    ]
    concat_zeros = [
        np.zeros((N_CORES * z.shape[0], *z.shape[1:]), z.dtype)
        for z in zero_outs
    ]
    sharding = jax.sharding.NamedSharding(mesh, PartitionSpec("core"))
    dev_args = [jax.device_put(a, sharding) for a in concat_in + concat_zeros]

    out = sharded(*dev_args)
    jax.block_until_ready(out)
    t0 = time.perf_counter()
    for _ in range(n_iters):
        out = sharded(*dev_args)
    jax.block_until_ready(out)
    t1 = time.perf_counter()
    per_iter_ns = (t1 - t0) / n_iters * 1e9
    result = np.asarray(out[0]).reshape(N_CORES * NB, L, C)
    return per_iter_ns, result



# revision 7
# speedup vs baseline: 1.4255x; 1.4255x over previous
"""CBAM kernel for Trainium2, 8-way batch-parallel SPMD.

Computes out = x^2 * (att_c[b,c] + sigmoid(conv(spatial_stats))[b,l]) where
att_c = sigmoid(mlp(mean_L x) + mlp(max_L x)), matching the CBAM reference.

Layout per core: 4 batches; each batch x[4096, 256] lives in SBUF as one
[128, 8192] tensor where partition p holds DRAM rows [32p, 32p+32) — i.e.
32 KiB of CONTIGUOUS DRAM per partition, so each half-batch load/store is
one 2 MiB DMA with 16 KiB descriptors (near-peak HBM efficiency).
l = 32*p + j, free column = 256*j + c, j in [0, 32).

Engine split per batch:
  ACT  : fp32->bf16 cast with spatial-sum accum_out (one op per j-block),
         squares (bf16 out), sigmoids, small copies
  DVE  : channel-max + spatial-max as bf16 TensorTensor fold trees (2x
         perf mode), 7-tap conv STT chain, final (att + sig) * x^2 STTs
  PE   : channel-sum (bf16 1/L-column matmul accumulation), c-major stats
         transposes, MLP, conv halo shifts
  POOL : SWDGE store queue only (bf16 -> fp32 cast during store DMA)
DMA: loads on sync (HWDGE) as 2 MiB transfers; stores on gpsimd (SWDGE,
casting) one batch late; output writes to HBM stay full fp32.
"""

import numpy as np
from contextlib import ExitStack

import concourse.bacc as bacc
import concourse.bass as bass
import concourse.tile as tile
import concourse.mybir as mybir
from concourse.bass_utils import run_bass_kernel_spmd

AF = mybir.ActivationFunctionType
ALU = mybir.AluOpType
AX = mybir.AxisListType
FP32 = mybir.dt.float32
BF16 = mybir.dt.bfloat16

N_CORES = 8
B_FULL = 32
NB = B_FULL // N_CORES  # batches per core = 4
L = 4096
C = 256
HID = 16
HB = HID + 1
P = 128
NJ = L // P  # 32 j-blocks (rows per partition)
HJ = NJ // 2  # 16 j-blocks per half
HALF = HJ * C  # 4096 free columns per half

_CACHE: dict = {}


def _build_body(ctx: ExitStack, tc, out_d, x_d, w1_d, b1_d, w2b_d, id_d,
                id16_d, ones_d, rc16_d, shup_d, shdn_d, wac_d, wmc_d, reps=1):
    nc = tc.nc

    const = ctx.enter_context(tc.tile_pool(name="const", bufs=1))
    xpool = ctx.enter_context(tc.tile_pool(name="x", bufs=2))
    bpool = ctx.enter_context(tc.tile_pool(name="xb16", bufs=2))
    opool = ctx.enter_context(tc.tile_pool(name="outt", bufs=2))
    sqpool = ctx.enter_context(tc.tile_pool(name="sq", bufs=3))
    mpool = ctx.enter_context(tc.tile_pool(name="maxtree", bufs=1))
    smpool = ctx.enter_context(tc.tile_pool(name="spattree", bufs=1))
    spool = ctx.enter_context(tc.tile_pool(name="stats", bufs=2))
    pacc = ctx.enter_context(tc.tile_pool(name="pacc", bufs=2, space="PSUM"))
    ptr = ctx.enter_context(tc.tile_pool(name="ptr", bufs=2, space="PSUM"))
    pwork = ctx.enter_context(tc.tile_pool(name="pwork", bufs=2, space="PSUM"))

    w1 = const.tile([P, 2 * HB], FP32)
    nc.gpsimd.dma_start(w1[:], w1_d[:])
    b1 = const.tile([HB, 1], FP32)
    nc.gpsimd.dma_start(b1[:], b1_d[:])
    w2b = const.tile([HB, C], FP32)
    nc.gpsimd.dma_start(w2b[:], w2b_d[:])
    ident = const.tile([P, P], FP32)
    nc.gpsimd.dma_start(ident[:], id_d[:])
    ident16 = const.tile([P, P], BF16)
    nc.gpsimd.dma_start(ident16[:], id16_d[:])
    ones = const.tile([P, P], FP32)
    nc.gpsimd.dma_start(ones[:], ones_d[:])
    redcol16 = const.tile([P, 1], BF16)
    nc.gpsimd.dma_start(redcol16[:], rc16_d[:])
    shup = const.tile([P, P], FP32)
    nc.gpsimd.dma_start(shup[:], shup_d[:])
    shdn = const.tile([P, P], FP32)
    nc.gpsimd.dma_start(shdn[:], shdn_d[:])
    wac = const.tile([P, 7], FP32)
    nc.gpsimd.dma_start(wac[:], wac_d[:])
    wmc = const.tile([P, 7], FP32)
    nc.gpsimd.dma_start(wmc[:], wmc_d[:])

    pending = []  # stores for the previous batch, flushed one batch late
    for b in [b for _ in range(reps) for b in range(NB)]:
        xt = xpool.tile([P, NJ * C], FP32, tag="x", name="x")
        xv = x_d[b, :, :].rearrange("(p q) c -> p (q c)", p=P)
        nc.sync.dma_start(xt[:, 0:HALF], xv[:, 0:HALF])
        nc.sync.dma_start(xt[:, HALF:2 * HALF], xv[:, HALF:2 * HALF])

        for dst, src in pending:
            nc.gpsimd.dma_start(dst, src)
        pending = []

        # se* hold spatial stats with a 3-col halo on both sides:
        # se[p, 3 + j] = stat(l = 32p + j); cols 0:3 / 35:38 come from the
        # neighbouring partitions (or zero at the batch edges).
        sej = spool.tile([P, 38], FP32, tag="sej", name="sej")
        semf = spool.tile([P, 38], FP32, tag="semf", name="semf")
        xb = bpool.tile([P, NJ * C], BF16, tag="xb", name="xb")
        pcs = pacc.tile([1, C], FP32, tag="pcs")
        for h in range(2):
            # ---- ACT: cast to bf16 + spatial sum via accum_out ----
            for jh in range(HJ):
                j = HJ * h + jh
                nc.scalar.activation(xb[:, C * j:C * (j + 1)],
                                     xt[:, C * j:C * (j + 1)],
                                     AF.Identity,
                                     accum_out=sej[:, 3 + j:4 + j])
            # ---- PE: channel sum over (p, j), bf16 rhs ----
            for jh in range(HJ):
                j = HJ * h + jh
                nc.tensor.matmul(pcs[:], redcol16[:],
                                 xb[:, C * j:C * (j + 1)],
                                 start=(j == 0), stop=(j == NJ - 1),
                                 skip_group_check=True)

        # ---- channel max over j (DVE): flat bf16 fold tree ----
        mh = mpool.tile([P, NJ * C // 2], BF16, tag="mh", name="mh")
        nc.vector.tensor_max(mh[:], xb[:, 0:HALF], xb[:, HALF:2 * HALF])
        w = HALF // 2
        while w >= C:
            nc.vector.tensor_max(mh[:, 0:w], mh[:, 0:w], mh[:, w:2 * w])
            w //= 2

        # ---- spatial max over c (DVE): strided bf16 fold tree ----
        sm = smpool.tile([P, NJ * (C // 2)], BF16, tag="sm", name="sm")
        sm3 = sm[:].rearrange("p (j c) -> p j c", c=C // 2)
        v3 = xb[:].rearrange("p (j c) -> p j c", c=C)
        nc.vector.tensor_max(sm3[:, :, :], v3[:, :, 0:C // 2],
                             v3[:, :, C // 2:C])
        w = C // 4
        while w >= 1:
            out = (semf[:, 3:3 + NJ].rearrange("p (j o) -> p j o", o=1)
                   if w == 1 else sm3[:, :, 0:w])
            nc.vector.tensor_max(out, sm3[:, :, 0:w], sm3[:, :, w:2 * w])
            w //= 2

        # ---- channel stats into c-major [128, 4] via PE transposes ----
        avg_row = spool.tile([1, C], FP32, tag="avg", name="avg")
        nc.scalar.activation(avg_row[:], pcs[:], AF.Copy)
        stats = spool.tile([P, 4], FP32, tag="stats", name="stats")
        pT = ptr.tile([P, 2], FP32, tag="pT")
        nc.tensor.transpose(pT[:, 0:1], avg_row[0:1, 0:P], ident[0:1, 0:1])
        nc.tensor.transpose(pT[:, 1:2], avg_row[0:1, P:C], ident[0:1, 0:1])
        mT = ptr.tile([P, 2 * P], BF16, tag="mT")
        nc.tensor.transpose(mT[:, 0:P], mh[:, 0:P], ident16[:, :])
        nc.tensor.transpose(mT[:, P:2 * P], mh[:, P:C], ident16[:, :])
        nc.scalar.copy(stats[:, 0:1], pT[:, 0:1])
        nc.scalar.copy(stats[:, 2:3], pT[:, 1:2])
        nc.vector.tensor_reduce(stats[:, 1:2], mT[:, 0:P],
                                axis=AX.X, op=ALU.max)
        nc.vector.tensor_reduce(stats[:, 3:4], mT[:, P:2 * P],
                                axis=AX.X, op=ALU.max)

        # ---- shared MLP: row HID carries the 2*b2 constant trick ----
        # one PSUM bank packs po 0:256, psh 256:268, ph 268:270
        wk = pwork.tile([P, C + 14], FP32, tag="wk")
        ph = wk[0:HB, C + 12:C + 14]
        nc.tensor.matmul(ph[:], w1[:, 0:HB], stats[:, 0:2],
                         start=True, stop=False, skip_group_check=True)
        nc.tensor.matmul(ph[:], w1[:, HB:2 * HB], stats[:, 2:4],
                         start=False, stop=True, skip_group_check=True)
        hsb = spool.tile([HB, 2], FP32, tag="hsb", name="hsb")
        nc.scalar.activation(hsb[:], ph[:], AF.Relu, bias=b1[:])
        h2 = spool.tile([HB, 1], FP32, tag="h2", name="h2")
        nc.vector.tensor_add(h2[:], hsb[:, 0:1], hsb[:, 1:2])
        h2r = spool.tile([HB, P], FP32, tag="h2r", name="h2r")
        nc.scalar.mul(h2r[:], ones[0:HB, :], h2[:])
        po = wk[:, 0:C]
        nc.tensor.matmul(po[:], h2r[:], w2b[:], start=True, stop=True,
                         skip_group_check=True)
        att = spool.tile([P, C], BF16, tag="att", name="att")
        nc.scalar.activation(att[:], po[:], AF.Sigmoid)

        # ---- conv halo: neighbour-partition stats via PE shift matmuls ----
        psh = wk[:, C:C + 12]
        nc.tensor.matmul(psh[:, 0:3], shup[:], sej[:, 32:35],
                         start=True, stop=True, skip_group_check=True)
        nc.tensor.matmul(psh[:, 3:6], shup[:], semf[:, 32:35],
                         start=True, stop=True, skip_group_check=True)
        nc.tensor.matmul(psh[:, 6:9], shdn[:], sej[:, 3:6],
                         start=True, stop=True, skip_group_check=True)
        nc.tensor.matmul(psh[:, 9:12], shdn[:], semf[:, 3:6],
                         start=True, stop=True, skip_group_check=True)
        nc.scalar.copy(sej[:, 0:3], psh[:, 0:3])
        nc.scalar.copy(semf[:, 0:3], psh[:, 3:6])
        nc.scalar.copy(sej[:, 35:38], psh[:, 6:9])
        nc.scalar.copy(semf[:, 35:38], psh[:, 9:12])

        # ---- 7-tap conv along j: ACT init then DVE STT ping-pong ----
        cva = spool.tile([P, NJ], FP32, tag="cva", name="cva")
        cvb = spool.tile([P, NJ], FP32, tag="cvb", name="cvb")
        nc.scalar.mul(cva[:], sej[:, 0:NJ], wac[:, 0:1])
        cur, nxt = cva, cvb
        for k in range(1, 7):
            nc.vector.scalar_tensor_tensor(nxt[:], sej[:, k:k + NJ],
                                           wac[:, k:k + 1], cur[:],
                                           op0=ALU.mult, op1=ALU.add)
            cur, nxt = nxt, cur
        for k in range(7):
            nc.vector.scalar_tensor_tensor(nxt[:], semf[:, k:k + NJ],
                                           wmc[:, k:k + 1], cur[:],
                                           op0=ALU.mult, op1=ALU.add)
            cur, nxt = nxt, cur
        sig = spool.tile([P, NJ], FP32, tag="sig", name="sig")
        nc.scalar.activation(sig[:], cur[:], AF.Sigmoid)

        # ---- final: out = (att + sig) * x^2 in bf16 (DVE STT) ----
        ot = opool.tile([P, NJ * C], BF16, tag="ot", name="ot")
        for s in range(4):
            sq = sqpool.tile([P, 8 * C], BF16, tag="sq")
            nc.scalar.activation(sq[:], xt[:, 8 * C * s:8 * C * (s + 1)],
                                 AF.Square)
            for j8 in range(8):
                j = 8 * s + j8
                nc.vector.scalar_tensor_tensor(ot[:, C * j:C * (j + 1)],
                                               att[:], sig[:, j:j + 1],
                                               sq[:, C * j8:C * (j8 + 1)],
                                               op0=ALU.add, op1=ALU.mult)
        ov = out_d[b, :, :].rearrange("(p q) c -> p (q c)", p=P)
        pending = [(ov[:, 0:HALF], ot[:, 0:HALF]),
                   (ov[:, HALF:2 * HALF], ot[:, HALF:2 * HALF])]

    for dst, src in pending:
        nc.gpsimd.dma_start(dst, src)


def _build_nc(reps=1):
    nc = bacc.Bacc("TRN2", target_bir_lowering=False, debug=False,
                   enable_asserts=False, num_devices=N_CORES)
    x_d = nc.dram_tensor("xin", [NB, L, C], FP32, kind="ExternalInput").ap()
    w1_d = nc.dram_tensor("w1sb", [P, 2 * HB], FP32, kind="ExternalInput").ap()
    b1_d = nc.dram_tensor("b1col", [HB, 1], FP32, kind="ExternalInput").ap()
    w2b_d = nc.dram_tensor("w2b", [HB, C], FP32, kind="ExternalInput").ap()
    id_d = nc.dram_tensor("ident", [P, P], FP32, kind="ExternalInput").ap()
    id16_d = nc.dram_tensor("ident16", [P, P], BF16, kind="ExternalInput").ap()
    ones_d = nc.dram_tensor("ones", [P, P], FP32, kind="ExternalInput").ap()
    rc16_d = nc.dram_tensor("redcol16", [P, 1], BF16, kind="ExternalInput").ap()
    shup_d = nc.dram_tensor("shup", [P, P], FP32, kind="ExternalInput").ap()
    shdn_d = nc.dram_tensor("shdn", [P, P], FP32, kind="ExternalInput").ap()
    wac_d = nc.dram_tensor("wac", [P, 7], FP32, kind="ExternalInput").ap()
    wmc_d = nc.dram_tensor("wmc", [P, 7], FP32, kind="ExternalInput").ap()
    out_d = nc.dram_tensor("out", [NB, L, C], FP32, kind="ExternalOutput").ap()

    with tile.TileContext(nc) as tc:
        with ExitStack() as ctx:
            _build_body(ctx, tc, out_d, x_d, w1_d, b1_d, w2b_d, id_d,
                        id16_d, ones_d, rc16_d, shup_d, shdn_d, wac_d,
                        wmc_d, reps=reps)
    nc.compile()
    return nc


def get_nc(reps=1):
    key = f"nc{reps}"
    if key not in _CACHE:
        _CACHE[key] = _build_nc(reps=reps)
    return _CACHE[key]


def _prep_inputs(W1, b1, W2, b2, conv_w):
    """Host-side parameter preprocessing (shared across cores)."""
    W1 = np.asarray(W1, np.float32)
    W2 = np.asarray(W2, np.float32)
    b1 = np.asarray(b1, np.float32)
    b2 = np.asarray(b2, np.float32)
    conv_w = np.asarray(conv_w, np.float32)

    w1sb = np.zeros((P, 2 * HB), np.float32)
    for h in range(2):
        w1sb[:, HB * h:HB * h + HID] = W1[P * h:P * (h + 1), :]
    w2b = np.concatenate([W2, b2[None, :]], axis=0).astype(np.float32)
    b1col = np.concatenate([b1, [1.0]]).astype(np.float32).reshape(HB, 1)

    # Conv taps broadcast down the partitions; avg tap folds in the 1/C
    # spatial-mean scale (device computes raw channel sums).
    wa = (conv_w[:, 0, 0] / C).astype(np.float32)
    wm = conv_w[:, 1, 0].astype(np.float32)
    wac = np.broadcast_to(wa[None, :], (P, 7)).copy()
    wmc = np.broadcast_to(wm[None, :], (P, 7)).copy()

    # shift matmul lhsTs: out[p, :] = rhs[p-1, :] needs lhsT[k, p] = d(k, p-1)
    shup = np.eye(P, P, 1, dtype=np.float32)
    shdn = np.eye(P, P, -1, dtype=np.float32)
    import ml_dtypes
    bf16 = ml_dtypes.bfloat16
    return {
        "w1sb": w1sb,
        "b1col": np.ascontiguousarray(b1col),
        "w2b": w2b,
        "ident": np.eye(P, dtype=np.float32),
        "ident16": np.eye(P, dtype=bf16),
        "ones": np.ones((P, P), np.float32),
        "redcol16": np.full((P, 1), 1.0 / L, bf16),
        "shup": shup,
        "shdn": shdn,
        "wac": wac,
        "wmc": wmc,
    }


def kernel(x, W1, b1, W2, b2, conv_w):
    nc = get_nc()
    x = np.asarray(x, np.float32)
    params = _prep_inputs(W1, b1, W2, b2, conv_w)
    in_maps = []
    for c in range(N_CORES):
        m = dict(params)
        m["xin"] = np.ascontiguousarray(x[NB * c:NB * (c + 1)])
        in_maps.append(m)
    _CACHE["last_in_maps"] = in_maps
    res = run_bass_kernel_spmd(nc, in_maps, list(range(N_CORES)))
    _CACHE["last_results"] = res
    return np.concatenate([res.results[c]["out"] for c in range(N_CORES)],
                          axis=0)


def _pjrt_exec(nc, in_maps, n_warm=2, n_time=8):
    """Build a sharded jit for nc, run it, return (best_wall_s, result)."""
    import time
    import jax
    import concourse.mybir as mybir_
    from concourse.bass2jax import (_bass_exec_p, install_neuronx_cc_hook,
                                    partition_id_tensor)
    from jax.experimental.shard_map import shard_map
    from jax.sharding import Mesh, PartitionSpec

    install_neuronx_cc_hook()
    partition_name = (nc.partition_id_tensor.name
                      if nc.partition_id_tensor else None)
    in_names, out_names, out_avals = [], [], []
    for alloc in nc.m.functions[0].allocations:
        if not isinstance(alloc, mybir_.MemoryLocationSet):
            continue
        name = alloc.memorylocations[0].name
        if alloc.kind == "ExternalInput":
            if name != partition_name:
                in_names.append(name)
        elif alloc.kind == "ExternalOutput":
            out_names.append(name)
            out_avals.append(jax.core.ShapedArray(
                tuple(alloc.tensor_shape), mybir_.dt.np(alloc.dtype)))
    n_params = len(in_names)
    all_in_names = list(in_names) + list(out_names)
    if partition_name is not None:
        all_in_names.append(partition_name)

    def _body(*args):
        operands = list(args)
        if partition_name is not None:
            operands.append(partition_id_tensor())
        return tuple(_bass_exec_p.bind(
            *operands,
            out_avals=tuple(out_avals),
            in_names=tuple(all_in_names),
            out_names=tuple(out_names),
            lowering_input_output_aliases=(),
            sim_require_finite=True,
            sim_require_nnan=True,
            nc=nc,
        ))

    devices = jax.devices()[:N_CORES]
    mesh = Mesh(np.asarray(devices), ("core",))
    nin = n_params + len(out_names)
    sharding = jax.sharding.NamedSharding(mesh, PartitionSpec("core"))
    fn = jax.jit(shard_map(
        _body, mesh=mesh,
        in_specs=(PartitionSpec("core"),) * nin,
        out_specs=(PartitionSpec("core"),) * len(out_names),
        check_rep=False))
    dev_args = [
        jax.device_put(np.concatenate(
            [np.asarray(in_maps[c][nm]) for c in range(N_CORES)], axis=0),
            sharding)
        for nm in in_names
    ]
    for av in out_avals:
        z = np.zeros((N_CORES * av.shape[0], *av.shape[1:]), av.dtype)
        dev_args.append(jax.device_put(z, sharding))

    for _ in range(n_warm):
        out = fn(*dev_args)
        jax.block_until_ready(out)
    best = float("inf")
    for _ in range(n_time):
        t0 = time.perf_counter()
        out = fn(*dev_args)
        jax.block_until_ready(out)
        best = min(best, time.perf_counter() - t0)
    result = np.asarray(out[0]).reshape(N_CORES * NB, L, C)
    return best, result


def bench_repeat(reps=8, n_time=10, in_maps=None):
    """Isolate device exec time: time a module doing the work `reps` times
    in-kernel vs once; slope = steady-state HW time per execution."""
    if in_maps is None:
        in_maps = _CACHE["last_in_maps"]
    t1, _ = _pjrt_exec(get_nc(1), in_maps, n_time=n_time)
    tr, result = _pjrt_exec(get_nc(reps), in_maps, n_time=n_time)
    per_exec_ns = (tr - t1) / (reps - 1) * 1e9
    return per_exec_ns, result, t1 * 1e9, tr * 1e9


def bench(n_iters=30, in_maps=None):
    """Time back-to-back NEFF executions with device-resident inputs."""
    import time
    import jax
    import concourse.mybir as mybir_
    from concourse.bass2jax import (_bass_exec_p, install_neuronx_cc_hook,
                                    partition_id_tensor)
    from jax.experimental.shard_map import shard_map
    from jax.sharding import Mesh, PartitionSpec

    nc = get_nc()
    if in_maps is None:
        in_maps = _CACHE["last_in_maps"]
    install_neuronx_cc_hook()

    partition_name = (nc.partition_id_tensor.name
                      if nc.partition_id_tensor else None)
    in_names, out_names, out_avals, zero_outs = [], [], [], []
    for alloc in nc.m.functions[0].allocations:
        if not isinstance(alloc, mybir_.MemoryLocationSet):
            continue
        name = alloc.memorylocations[0].name
        if alloc.kind == "ExternalInput":
            if name != partition_name:
                in_names.append(name)
        elif alloc.kind == "ExternalOutput":
            shape = tuple(alloc.tensor_shape)
            dtype = mybir_.dt.np(alloc.dtype)
            out_names.append(name)
            out_avals.append(jax.core.ShapedArray(shape, dtype))
            zero_outs.append(np.zeros(shape, dtype))
    n_params = len(in_names)
    all_in_names = list(in_names) + list(out_names)
    if partition_name is not None:
        all_in_names.append(partition_name)

    def _body(*args):
        operands = list(args)
        if partition_name is not None:
            operands.append(partition_id_tensor())
        return tuple(_bass_exec_p.bind(
            *operands,
            out_avals=tuple(out_avals),
            in_names=tuple(all_in_names),
            out_names=tuple(out_names),
            lowering_input_output_aliases=(),
            sim_require_finite=True,
            sim_require_nnan=True,
            nc=nc,
        ))

    devices = jax.devices()[:N_CORES]
    mesh = Mesh(np.asarray(devices), ("core",))
    nin = n_params + len(out_names)
    sharded = jax.jit(shard_map(
        _body, mesh=mesh,
        in_specs=(PartitionSpec("core"),) * nin,
        out_specs=(PartitionSpec("core"),) * len(out_names),
        check_rep=False))

    concat_in = [
        np.concatenate([np.asarray(in_maps[c][nm]) for c in range(N_CORES)],
                       axis=0)
        for nm in in_names
    ]
    concat_zeros = [
        np.zeros((N_CORES * z.shape[0], *z.shape[1:]), z.dtype)
        for z in zero_outs
    ]
    sharding = jax.sharding.NamedSharding(mesh, PartitionSpec("core"))
    dev_args = [jax.device_put(a, sharding) for a in concat_in + concat_zeros]

    out = sharded(*dev_args)
    jax.block_until_ready(out)
    t0 = time.perf_counter()
    for _ in range(n_iters):
        out = sharded(*dev_args)
    jax.block_until_ready(out)
    t1 = time.perf_counter()
    per_iter_ns = (t1 - t0) / n_iters * 1e9
    result = np.asarray(out[0]).reshape(N_CORES * NB, L, C)
    return per_iter_ns, result


# revision 9
# speedup vs baseline: 1.5343x; 1.0763x over previous
"""CBAM kernel for Trainium2, 8-way batch-parallel SPMD.

Computes out = x^2 * (att_c[b,c] + sigmoid(conv(spatial_stats))[b,l]) where
att_c = sigmoid(mlp(mean_L x) + mlp(max_L x)), matching the CBAM reference.

Layout per core: 4 batches; each batch x[4096, 256] lives in SBUF as one
[128, 8192] bf16 tensor (partition p holds DRAM rows [32p, 32p+32), i.e.
32 KiB contiguous fp32 DRAM per partition; the fp32->bf16 cast happens for
free inside the SWDGE load DMA). l = 32*p + j, free col = 256*j + c.

Engine split per batch (~23 us DMA window):
  DVE  : channel-max + spatial-max bf16 fold trees (2x perf mode), 8-j
         spatial-sum tree, the 32 final (att+sig)*x^2 STTs (deferred one
         batch so they fill the next load window), small reduces
  ACT  : 24-j spatial-sum accumulator copies, squares (bf16), sigmoids,
         PSUM->SBUF copies
  PE   : channel-sum (16x [128,512] bf16 matmuls), stats transposes, MLP,
         7-tap conv as banded-Toeplitz matmuls in transposed [j, p] space
         (corner taps = column-shifted rhs, no halo exchange needed)
  POOL : SWDGE DMA queue only - cast loads, cast stores (bf16 -> fp32)
"""

import numpy as np
from contextlib import ExitStack

import concourse.bacc as bacc
import concourse.bass as bass
import concourse.tile as tile
import concourse.mybir as mybir
from concourse.bass_utils import run_bass_kernel_spmd

AF = mybir.ActivationFunctionType
ALU = mybir.AluOpType
AX = mybir.AxisListType
FP32 = mybir.dt.float32
BF16 = mybir.dt.bfloat16

N_CORES = 8
B_FULL = 32
NB = B_FULL // N_CORES  # batches per core = 4
L = 4096
C = 256
HID = 16
HB = HID + 1
P = 128
NJ = L // P  # 32 j-blocks (rows per partition)
HJ = NJ // 2  # 16 j-blocks per half
HALF = HJ * C  # 4096 free columns per half

_CACHE: dict = {}


def _build_body(ctx: ExitStack, tc, out_d, x_d, w1_d, b1_d, w2b_d, id_d,
                id16_d, ones_d, rc16_d, tj_d, reps=1):
    nc = tc.nc

    const = ctx.enter_context(tc.tile_pool(name="const", bufs=1))
    bpool = ctx.enter_context(tc.tile_pool(name="xb16", bufs=2))
    opool = ctx.enter_context(tc.tile_pool(name="outt", bufs=2))
    sqpool = ctx.enter_context(tc.tile_pool(name="sq", bufs=8))
    mpool = ctx.enter_context(tc.tile_pool(name="maxtree", bufs=1))
    smpool = ctx.enter_context(tc.tile_pool(name="spattree", bufs=1))
    spool = ctx.enter_context(tc.tile_pool(name="stats", bufs=2))
    dpool = ctx.enter_context(tc.tile_pool(name="dummy", bufs=2))
    pacc = ctx.enter_context(tc.tile_pool(name="pacc", bufs=2, space="PSUM"))
    pwk = ctx.enter_context(tc.tile_pool(name="pwk", bufs=2, space="PSUM"))
    pcnv = ctx.enter_context(tc.tile_pool(name="pcnv", bufs=2, space="PSUM"))
    pb16 = ctx.enter_context(tc.tile_pool(name="pb16", bufs=2, space="PSUM"))

    w1 = const.tile([P, 2 * HB], FP32)
    nc.gpsimd.dma_start(w1[:], w1_d[:])
    b1 = const.tile([HB, 1], FP32)
    nc.gpsimd.dma_start(b1[:], b1_d[:])
    w2b = const.tile([HB, C], FP32)
    nc.gpsimd.dma_start(w2b[:], w2b_d[:])
    ident = const.tile([P, P], FP32)
    nc.gpsimd.dma_start(ident[:], id_d[:])
    ident16 = const.tile([P, P], BF16)
    nc.gpsimd.dma_start(ident16[:], id16_d[:])
    ones = const.tile([P, P], FP32)
    nc.gpsimd.dma_start(ones[:], ones_d[:])
    redcol16 = const.tile([P, 1], BF16)
    nc.gpsimd.dma_start(redcol16[:], rc16_d[:])
    tj = const.tile([NJ, 6 * NJ], FP32)
    nc.gpsimd.dma_start(tj[:], tj_d[:])

    NSA = 24  # j-blocks whose spatial sum rides ACT accumulator copies

    def emit_final(prev):
        """Final combine (att + sig) * x^2 for the previous batch + stores.

        Emitted at the top of the next iteration so the 32 DVE STTs fill
        the load window of the current batch, and the stores land on the
        SWDGE queue right behind the current batch's loads."""
        att, psig, sqs, pb = prev
        ot = opool.tile([P, NJ * C], FP32, tag="ot", name="ot")
        for s in range(4):
            for j8 in range(8):
                j = 8 * s + j8
                nc.vector.scalar_tensor_tensor(ot[:, C * j:C * (j + 1)],
                                               att[:], psig[:, j:j + 1],
                                               sqs[s][:, C * j8:C * (j8 + 1)],
                                               op0=ALU.add, op1=ALU.mult)
        ov = out_d[pb, :, :].rearrange("(p q) c -> p (q c)", p=P)
        nc.sync.dma_start(ov[:, 0:HALF], ot[:, 0:HALF])
        nc.sync.dma_start(ov[:, HALF:2 * HALF], ot[:, HALF:2 * HALF])

    prev = None
    for b in [b for _ in range(reps) for b in range(NB)]:
        xb = bpool.tile([P, NJ * C], BF16, tag="xb", name="xb")
        xv = x_d[b, :, :].rearrange("(p q) c -> p (q c)", p=P)
        nc.gpsimd.dma_start(xb[:, 0:HALF], xv[:, 0:HALF])
        nc.gpsimd.dma_start(xb[:, HALF:2 * HALF], xv[:, HALF:2 * HALF])

        if prev is not None:
            emit_final(prev)

        sej = spool.tile([P, NJ], FP32, tag="sej", name="sej")
        semf = spool.tile([P, NJ], FP32, tag="semf", name="semf")
        pcs = pacc.tile([1, 2 * C], FP32, tag="pcs")
        sqs = []
        for h in range(2):
            # ---- ACT: spatial-sum accumulator copies (j < NSA) ----
            for jh in range(HJ):
                j = HJ * h + jh
                if j < NSA:
                    dummy = dpool.tile([P, C], BF16, tag="dummy")
                    nc.scalar.activation(dummy[:], xb[:, C * j:C * (j + 1)],
                                         AF.Identity,
                                         accum_out=sej[:, j:j + 1])
            # ---- ACT: squares (feed next iteration's final combine) ----
            for q in range(2):
                sq = sqpool.tile([P, 8 * C], BF16, tag="sq")
                off = (2 * h + q) * 8 * C
                nc.scalar.activation(sq[:], xb[:, off:off + 8 * C], AF.Square)
                sqs.append(sq)
            # ---- PE: channel sum, 8x [128, 512] bf16 matmuls per half ----
            for m8 in range(8):
                m = 8 * h + m8
                nc.tensor.matmul(pcs[:], redcol16[:],
                                 xb[:, 512 * m:512 * (m + 1)],
                                 start=(m == 0), stop=(m == 15),
                                 skip_group_check=True)

        # ---- DVE: channel max over j, flat bf16 fold tree ----
        mh = mpool.tile([P, NJ * C // 2], BF16, tag="mh", name="mh")
        nc.vector.tensor_max(mh[:], xb[:, 0:HALF], xb[:, HALF:2 * HALF])
        w = HALF // 2
        while w >= C:
            nc.vector.tensor_max(mh[:, 0:w], mh[:, 0:w], mh[:, w:2 * w])
            w //= 2

        # ---- DVE: spatial max over c, strided bf16 fold tree ----
        sm = smpool.tile([P, NJ * (C // 2)], BF16, tag="sm", name="sm")
        sm3 = sm[:].rearrange("p (j c) -> p j c", c=C // 2)
        v3 = xb[:].rearrange("p (j c) -> p j c", c=C)
        nc.vector.tensor_max(sm3[:, :, :], v3[:, :, 0:C // 2],
                             v3[:, :, C // 2:C])
        w = C // 4
        while w >= 1:
            out = (semf[:, :].rearrange("p (j o) -> p j o", o=1)
                   if w == 1 else sm3[:, :, 0:w])
            nc.vector.tensor_max(out, sm3[:, :, 0:w], sm3[:, :, w:2 * w])
            w //= 2

        # ---- DVE: spatial sum for j >= NSA, small bf16 add tree ----
        nrem = NJ - NSA
        ss = smpool.tile([P, nrem * (C // 2)], BF16, tag="ss", name="ss")
        ss3 = ss[:].rearrange("p (j c) -> p j c", c=C // 2)
        v8 = xb[:, C * NSA:C * NJ].rearrange("p (j c) -> p j c", c=C)
        nc.vector.tensor_add(ss3[:, :, :], v8[:, :, 0:C // 2],
                             v8[:, :, C // 2:C])
        w = C // 4
        while w >= 1:
            out = (sej[:, NSA:NJ].rearrange("p (j o) -> p j o", o=1)
                   if w == 1 else ss3[:, :, 0:w])
            nc.vector.tensor_add(out, ss3[:, :, 0:w], ss3[:, :, w:2 * w])
            w //= 2

        # ---- channel stats into c-major [128, 4] via PE transposes ----
        avgw = spool.tile([1, 2 * C], FP32, tag="avgw", name="avgw")
        nc.scalar.copy(avgw[:], pcs[0:1, :])
        avg_row = spool.tile([1, C], FP32, tag="avg", name="avg")
        nc.vector.tensor_add(avg_row[:], avgw[0:1, 0:C], avgw[0:1, C:2 * C])
        stats = spool.tile([P, 4], FP32, tag="stats", name="stats")
        # fp32 PSUM bank: po 0:256, ph 256:258, pT 258:260
        wk = pwk.tile([P, C + 4], FP32, tag="wk")
        pT = wk[:, C + 2:C + 4]
        nc.tensor.transpose(pT[:, 0:1], avg_row[0:1, 0:P], ident[0:1, 0:1])
        nc.tensor.transpose(pT[:, 1:2], avg_row[0:1, P:C], ident[0:1, 0:1])
        # bf16 PSUM bank: mT 0:256, sig 256:288
        mtp = pb16.tile([P, 2 * P + NJ], BF16, tag="mtp")
        mT = mtp[:, 0:2 * P]
        nc.tensor.transpose(mT[:, 0:P], mh[:, 0:P], ident16[:, :])
        nc.tensor.transpose(mT[:, P:2 * P], mh[:, P:C], ident16[:, :])
        nc.scalar.copy(stats[:, 0:1], pT[:, 0:1])
        nc.scalar.copy(stats[:, 2:3], pT[:, 1:2])
        nc.vector.tensor_reduce(stats[:, 1:2], mT[:, 0:P],
                                axis=AX.X, op=ALU.max)
        nc.vector.tensor_reduce(stats[:, 3:4], mT[:, P:2 * P],
                                axis=AX.X, op=ALU.max)

        # ---- shared MLP: row HID carries the 2*b2 constant trick ----
        ph = wk[0:HB, C:C + 2]
        nc.tensor.matmul(ph[:], w1[:, 0:HB], stats[:, 0:2],
                         start=True, stop=False, skip_group_check=True)
        nc.tensor.matmul(ph[:], w1[:, HB:2 * HB], stats[:, 2:4],
                         start=False, stop=True, skip_group_check=True)
        hsb = spool.tile([HB, 2], FP32, tag="hsb", name="hsb")
        nc.scalar.activation(hsb[:], ph[:], AF.Relu, bias=b1[:])
        h2 = spool.tile([HB, 1], FP32, tag="h2", name="h2")
        nc.vector.tensor_add(h2[:], hsb[:, 0:1], hsb[:, 1:2])
        h2r = spool.tile([HB, P], FP32, tag="h2r", name="h2r")
        nc.scalar.mul(h2r[:], ones[0:HB, :], h2[:])
        po = wk[:, 0:C]
        nc.tensor.matmul(po[:], h2r[:], w2b[:], start=True, stop=True,
                         skip_group_check=True)
        att = spool.tile([P, C], BF16, tag="att", name="att")
        nc.scalar.activation(att[:], po[:], AF.Sigmoid)

        # ---- 7-tap conv in transposed [j, p] space (PE Toeplitz) ----
        # fp32 PSUM bank: sjT 0:128, smT 128:256, pcv 256:384
        pct = pcnv.tile([NJ, 3 * P], FP32, tag="pct")
        nc.tensor.transpose(pct[:, 0:P], sej[:], ident[:, :])
        nc.tensor.transpose(pct[:, P:2 * P], semf[:], ident[:, :])
        sjS = spool.tile([NJ, P], FP32, tag="sjS", name="sjS")
        nc.scalar.copy(sjS[:], pct[:, 0:P])
        smS = spool.tile([NJ, P], FP32, tag="smS", name="smS")
        nc.scalar.copy(smS[:], pct[:, P:2 * P])
        pcv = pct[:, 2 * P:3 * P]
        nc.tensor.matmul(pcv[:, :], tj[:, 0:NJ], sjS[:, :],
                         start=True, stop=False, skip_group_check=True)
        nc.tensor.matmul(pcv[:, 1:P], tj[:, NJ:2 * NJ], sjS[:, 0:P - 1],
                         start=False, stop=False, skip_group_check=True)
        nc.tensor.matmul(pcv[:, 0:P - 1], tj[:, 2 * NJ:3 * NJ], sjS[:, 1:P],
                         start=False, stop=False, skip_group_check=True)
        nc.tensor.matmul(pcv[:, :], tj[:, 3 * NJ:4 * NJ], smS[:, :],
                         start=False, stop=False, skip_group_check=True)
        nc.tensor.matmul(pcv[:, 1:P], tj[:, 4 * NJ:5 * NJ], smS[:, 0:P - 1],
                         start=False, stop=False, skip_group_check=True)
        nc.tensor.matmul(pcv[:, 0:P - 1], tj[:, 5 * NJ:6 * NJ], smS[:, 1:P],
                         start=False, stop=True, skip_group_check=True)
        sigT = spool.tile([NJ, P], BF16, tag="sigT", name="sigT")
        nc.scalar.activation(sigT[:], pcv[:], AF.Sigmoid)
        psig = mtp[:, 2 * P:2 * P + NJ]
        nc.tensor.transpose(psig[:], sigT[:], ident16[0:NJ, 0:NJ])

        prev = (att, psig, sqs, b)

    emit_final(prev)


def _build_nc(reps=1):
    nc = bacc.Bacc("TRN2", target_bir_lowering=False, debug=False,
                   enable_asserts=False, num_devices=N_CORES)
    x_d = nc.dram_tensor("xin", [NB, L, C], FP32, kind="ExternalInput").ap()
    w1_d = nc.dram_tensor("w1sb", [P, 2 * HB], FP32, kind="ExternalInput").ap()
    b1_d = nc.dram_tensor("b1col", [HB, 1], FP32, kind="ExternalInput").ap()
    w2b_d = nc.dram_tensor("w2b", [HB, C], FP32, kind="ExternalInput").ap()
    id_d = nc.dram_tensor("ident", [P, P], FP32, kind="ExternalInput").ap()
    id16_d = nc.dram_tensor("ident16", [P, P], BF16, kind="ExternalInput").ap()
    ones_d = nc.dram_tensor("ones", [P, P], FP32, kind="ExternalInput").ap()
    rc16_d = nc.dram_tensor("redcol16", [P, 1], BF16, kind="ExternalInput").ap()
    tj_d = nc.dram_tensor("tjconv", [NJ, 6 * NJ], FP32, kind="ExternalInput").ap()
    out_d = nc.dram_tensor("out", [NB, L, C], FP32, kind="ExternalOutput").ap()

    with tile.TileContext(nc) as tc:
        with ExitStack() as ctx:
            _build_body(ctx, tc, out_d, x_d, w1_d, b1_d, w2b_d, id_d,
                        id16_d, ones_d, rc16_d, tj_d, reps=reps)
    nc.compile()
    return nc


def get_nc(reps=1):
    key = f"nc{reps}"
    if key not in _CACHE:
        _CACHE[key] = _build_nc(reps=reps)
    return _CACHE[key]


def _prep_inputs(W1, b1, W2, b2, conv_w):
    """Host-side parameter preprocessing (shared across cores)."""
    W1 = np.asarray(W1, np.float32)
    W2 = np.asarray(W2, np.float32)
    b1 = np.asarray(b1, np.float32)
    b2 = np.asarray(b2, np.float32)
    conv_w = np.asarray(conv_w, np.float32)

    w1sb = np.zeros((P, 2 * HB), np.float32)
    for h in range(2):
        w1sb[:, HB * h:HB * h + HID] = W1[P * h:P * (h + 1), :]
    w2b = np.concatenate([W2, b2[None, :]], axis=0).astype(np.float32)
    b1col = np.concatenate([b1, [1.0]]).astype(np.float32).reshape(HB, 1)

    # Transposed-space conv Toeplitz lhsTs [j', j]; the avg tap folds in the
    # 1/C spatial-mean scale (device computes raw channel sums).
    wa = (conv_w[:, 0, 0] / C).astype(np.float32)
    wm = conv_w[:, 1, 0].astype(np.float32)
    tj = np.zeros((NJ, 6 * NJ), np.float32)
    for jp in range(NJ):
        for j in range(NJ):
            k = jp - j + 3          # main band
            if 0 <= k < 7:
                tj[jp, j] = wa[k]
                tj[jp, 3 * NJ + j] = wm[k]
            k = jp - j - 29         # prev-partition corner
            if 0 <= k < 7 and jp >= 29 and j <= 2:
                tj[jp, NJ + j] = wa[k]
                tj[jp, 4 * NJ + j] = wm[k]
            k = jp + 35 - j         # next-partition corner
            if 0 <= k < 7 and jp <= 2 and j >= 29:
                tj[jp, 2 * NJ + j] = wa[k]
                tj[jp, 5 * NJ + j] = wm[k]

    import ml_dtypes
    bf16 = ml_dtypes.bfloat16
    return {
        "w1sb": w1sb,
        "b1col": np.ascontiguousarray(b1col),
        "w2b": w2b,
        "ident": np.eye(P, dtype=np.float32),
        "ident16": np.eye(P, dtype=bf16),
        "ones": np.ones((P, P), np.float32),
        "redcol16": np.full((P, 1), 1.0 / L, bf16),
        "tjconv": tj,
    }


def kernel(x, W1, b1, W2, b2, conv_w):
    nc = get_nc()
    x = np.asarray(x, np.float32)
    params = _prep_inputs(W1, b1, W2, b2, conv_w)
    in_maps = []
    for c in range(N_CORES):
        m = dict(params)
        m["xin"] = np.ascontiguousarray(x[NB * c:NB * (c + 1)])
        in_maps.append(m)
    _CACHE["last_in_maps"] = in_maps
    res = run_bass_kernel_spmd(nc, in_maps, list(range(N_CORES)))
    _CACHE["last_results"] = res
    return np.concatenate([res.results[c]["out"] for c in range(N_CORES)],
                          axis=0)


def _pjrt_exec(nc, in_maps, n_warm=2, n_time=8):
    """Build a sharded jit for nc, run it, return (best_wall_s, result)."""
    import time
    import jax
    import concourse.mybir as mybir_
    from concourse.bass2jax import (_bass_exec_p, install_neuronx_cc_hook,
                                    partition_id_tensor)
    from jax.experimental.shard_map import shard_map
    from jax.sharding import Mesh, PartitionSpec

    install_neuronx_cc_hook()
    partition_name = (nc.partition_id_tensor.name
                      if nc.partition_id_tensor else None)
    in_names, out_names, out_avals = [], [], []
    for alloc in nc.m.functions[0].allocations:
        if not isinstance(alloc, mybir_.MemoryLocationSet):
            continue
        name = alloc.memorylocations[0].name
        if alloc.kind == "ExternalInput":
            if name != partition_name:
                in_names.append(name)
        elif alloc.kind == "ExternalOutput":
            out_names.append(name)
            out_avals.append(jax.core.ShapedArray(
                tuple(alloc.tensor_shape), mybir_.dt.np(alloc.dtype)))
    n_params = len(in_names)
    all_in_names = list(in_names) + list(out_names)
    if partition_name is not None:
        all_in_names.append(partition_name)

    def _body(*args):
        operands = list(args)
        if partition_name is not None:
            operands.append(partition_id_tensor())
        return tuple(_bass_exec_p.bind(
            *operands,
            out_avals=tuple(out_avals),
            in_names=tuple(all_in_names),
            out_names=tuple(out_names),
            lowering_input_output_aliases=(),
            sim_require_finite=True,
            sim_require_nnan=True,
            nc=nc,
        ))

    devices = jax.devices()[:N_CORES]
    mesh = Mesh(np.asarray(devices), ("core",))
    nin = n_params + len(out_names)
    sharding = jax.sharding.NamedSharding(mesh, PartitionSpec("core"))
    fn = jax.jit(shard_map(
        _body, mesh=mesh,
        in_specs=(PartitionSpec("core"),) * nin,
        out_specs=(PartitionSpec("core"),) * len(out_names),
        check_rep=False))
    dev_args = [
        jax.device_put(np.concatenate(
            [np.asarray(in_maps[c][nm]) for c in range(N_CORES)], axis=0),
            sharding)
        for nm in in_names
    ]
    for av in out_avals:
        z = np.zeros((N_CORES * av.shape[0], *av.shape[1:]), av.dtype)
        dev_args.append(jax.device_put(z, sharding))

    for _ in range(n_warm):
        out = fn(*dev_args)
        jax.block_until_ready(out)
    best = float("inf")
    for _ in range(n_time):
        t0 = time.perf_counter()
        out = fn(*dev_args)
        jax.block_until_ready(out)
        best = min(best, time.perf_counter() - t0)
    result = np.asarray(out[0]).reshape(N_CORES * NB, L, C)
    return best, result


def bench_repeat(reps=8, n_time=10, in_maps=None):
    """Isolate device exec time: time a module doing the work `reps` times
    in-kernel vs once; slope = steady-state HW time per execution."""
    if in_maps is None:
        in_maps = _CACHE["last_in_maps"]
    t1, _ = _pjrt_exec(get_nc(1), in_maps, n_time=n_time)
    tr, result = _pjrt_exec(get_nc(reps), in_maps, n_time=n_time)
    per_exec_ns = (tr - t1) / (reps - 1) * 1e9
    return per_exec_ns, result, t1 * 1e9, tr * 1e9


def bench(n_iters=30, in_maps=None):
    """Time back-to-back NEFF executions with device-resident inputs."""
    import time
    import jax
    import concourse.mybir as mybir_
    from concourse.bass2jax import (_bass_exec_p, install_neuronx_cc_hook,
                                    partition_id_tensor)
    from jax.experimental.shard_map import shard_map
    from jax.sharding import Mesh, PartitionSpec

    nc = get_nc()
    if in_maps is None:
        in_maps = _CACHE["last_in_maps"]
    install_neuronx_cc_hook()

    partition_name = (nc.partition_id_tensor.name
                      if nc.partition_id_tensor else None)
    in_names, out_names, out_avals, zero_outs = [], [], [], []
    for alloc in nc.m.functions[0].allocations:
        if not isinstance(alloc, mybir_.MemoryLocationSet):
            continue
        name = alloc.memorylocations[0].name
        if alloc.kind == "ExternalInput":
            if name != partition_name:
                in_names.append(name)
        elif alloc.kind == "ExternalOutput":
            shape = tuple(alloc.tensor_shape)
            dtype = mybir_.dt.np(alloc.dtype)
            out_names.append(name)
            out_avals.append(jax.core.ShapedArray(shape, dtype))
            zero_outs.append(np.zeros(shape, dtype))
    n_params = len(in_names)
    all_in_names = list(in_names) + list(out_names)
    if partition_name is not None:
        all_in_names.append(partition_name)

    def _body(*args):
        operands = list(args)
        if partition_name is not None:
            operands.append(partition_id_tensor())
        return tuple(_bass_exec_p.bind(
            *operands,
            out_avals=tuple(out_avals),
            in_names=tuple(all_in_names),
            out_names=tuple(out_names),
            lowering_input_output_aliases=(),
            sim_require_finite=True,
            sim_require_nnan=True,
            nc=nc,
        ))

    devices = jax.devices()[:N_CORES]
    mesh = Mesh(np.asarray(devices), ("core",))
    nin = n_params + len(out_names)
    sharded = jax.jit(shard_map(
        _body, mesh=mesh,
        in_specs=(PartitionSpec("core"),) * nin,
        out_specs=(PartitionSpec("core"),) * len(out_names),
        check_rep=False))

    concat_in = [
        np.concatenate([np.asarray(in_maps[c][nm]) for c in range(N_CORES)],
                       axis=0)
        for nm in in_names
    ]
    concat_zeros = [
        np.zeros((N_CORES * z.shape[0], *z.shape[1:]), z.dtype)
        for z in zero_outs
    ]
    sharding = jax.sharding.NamedSharding(mesh, PartitionSpec("core"))
    dev_args = [jax.device_put(a, sharding) for a in concat_in + concat_zeros]

    out = sharded(*dev_args)
    jax.block_until_ready(out)
    t0 = time.perf_counter()
    for _ in range(n_iters):
        out = sharded(*dev_args)
    jax.block_until_ready(out)
    t1 = time.perf_counter()
    per_iter_ns = (t1 - t0) / n_iters * 1e9
    result = np.asarray(out[0]).reshape(N_CORES * NB, L, C)
    return per_iter_ns, result


# revision 10
# speedup vs baseline: 1.5700x; 1.0233x over previous
"""CBAM kernel for Trainium2, 8-way batch-parallel SPMD.

Computes out = x^2 * (att_c[b,c] + sigmoid(conv(spatial_stats))[b,l]) where
att_c = sigmoid(mlp(mean_L x) + mlp(max_L x)), matching the CBAM reference.

Layout per core: 4 batches; each batch x[4096, 256] lives in SBUF as one
[128, 8192] bf16 tensor (partition p holds DRAM rows [32p, 32p+32), i.e.
32 KiB contiguous fp32 DRAM per partition; the fp32->bf16 cast happens for
free inside the SWDGE load DMA). l = 32*p + j, free col = 256*j + c.

Engine split per batch (~23 us DMA window):
  DVE  : channel-max + spatial-max bf16 fold trees (2x perf mode), 8-j
         spatial-sum tree, the 32 final (att+sig)*x^2 STTs (deferred one
         batch so they fill the next load window), small reduces
  ACT  : 24-j spatial-sum accumulator copies, squares (bf16), sigmoids,
         PSUM->SBUF copies
  PE   : channel-sum (16x [128,512] bf16 matmuls), stats transposes, MLP,
         7-tap conv as banded-Toeplitz matmuls in transposed [j, p] space
         (corner taps = column-shifted rhs, no halo exchange needed)
  POOL : SWDGE DMA queue only - cast loads, cast stores (bf16 -> fp32)
"""

import numpy as np
from contextlib import ExitStack

import concourse.bacc as bacc
import concourse.bass as bass
import concourse.tile as tile
import concourse.mybir as mybir
from concourse.bass_utils import run_bass_kernel_spmd

AF = mybir.ActivationFunctionType
ALU = mybir.AluOpType
AX = mybir.AxisListType
FP32 = mybir.dt.float32
BF16 = mybir.dt.bfloat16

N_CORES = 8
B_FULL = 32
NB = B_FULL // N_CORES  # batches per core = 4
L = 4096
C = 256
HID = 16
HB = HID + 1
P = 128
NJ = L // P  # 32 j-blocks (rows per partition)
HJ = NJ // 2  # 16 j-blocks per half
HALF = HJ * C  # 4096 free columns per half

_CACHE: dict = {}


def _build_body(ctx: ExitStack, tc, out_d, x_d, w1_d, b1_d, w2b_d, id_d,
                id16_d, ones_d, rc16_d, tj_d, reps=1):
    nc = tc.nc

    const = ctx.enter_context(tc.tile_pool(name="const", bufs=1))
    bpool = ctx.enter_context(tc.tile_pool(name="xb16", bufs=2))
    opool = ctx.enter_context(tc.tile_pool(name="outt", bufs=2))
    sqpool = ctx.enter_context(tc.tile_pool(name="sq", bufs=8))
    mpool = ctx.enter_context(tc.tile_pool(name="maxtree", bufs=1))
    smpool = ctx.enter_context(tc.tile_pool(name="spattree", bufs=1))
    spool = ctx.enter_context(tc.tile_pool(name="stats", bufs=2))
    dpool = ctx.enter_context(tc.tile_pool(name="dummy", bufs=2))
    pacc = ctx.enter_context(tc.tile_pool(name="pacc", bufs=2, space="PSUM"))
    pwk = ctx.enter_context(tc.tile_pool(name="pwk", bufs=2, space="PSUM"))
    pcnv = ctx.enter_context(tc.tile_pool(name="pcnv", bufs=2, space="PSUM"))
    pb16 = ctx.enter_context(tc.tile_pool(name="pb16", bufs=2, space="PSUM"))

    w1 = const.tile([P, 2 * HB], FP32)
    nc.sync.dma_start(w1[:], w1_d[:])
    b1 = const.tile([HB, 1], FP32)
    nc.sync.dma_start(b1[:], b1_d[:])
    w2b = const.tile([HB, C], FP32)
    nc.sync.dma_start(w2b[:], w2b_d[:])
    ident = const.tile([P, P], FP32)
    nc.sync.dma_start(ident[:], id_d[:])
    ident16 = const.tile([P, P], BF16)
    nc.sync.dma_start(ident16[:], id16_d[:])
    ones = const.tile([P, P], FP32)
    nc.sync.dma_start(ones[:], ones_d[:])
    redcol16 = const.tile([P, 1], BF16)
    nc.sync.dma_start(redcol16[:], rc16_d[:])
    tj = const.tile([NJ, 6 * NJ], FP32)
    nc.sync.dma_start(tj[:], tj_d[:])

    NSA = 20  # j-blocks whose spatial sum rides ACT accumulator copies

    def emit_final(prev):
        """Final combine (att + sig) * x^2 for the previous batch + stores.

        Emitted at the top of the next iteration so the 32 DVE STTs fill
        the load window of the current batch, and the stores land on the
        SWDGE queue right behind the current batch's loads."""
        att, psig, sqs, pb = prev
        ot = opool.tile([P, NJ * C], FP32, tag="ot", name="ot")
        for s in range(4):
            for j8 in range(8):
                j = 8 * s + j8
                nc.vector.scalar_tensor_tensor(ot[:, C * j:C * (j + 1)],
                                               att[:], psig[:, j:j + 1],
                                               sqs[s][:, C * j8:C * (j8 + 1)],
                                               op0=ALU.add, op1=ALU.mult)
        ov = out_d[pb, :, :].rearrange("(p q) c -> p (q c)", p=P)
        nc.sync.dma_start(ov[:, 0:HALF], ot[:, 0:HALF])
        nc.sync.dma_start(ov[:, HALF:2 * HALF], ot[:, HALF:2 * HALF])

    prev = None
    for b in [b for _ in range(reps) for b in range(NB)]:
        xb = bpool.tile([P, NJ * C], BF16, tag="xb", name="xb")
        xv = x_d[b, :, :].rearrange("(p q) c -> p (q c)", p=P)
        nc.gpsimd.dma_start(xb[:, 0:HALF], xv[:, 0:HALF])
        nc.gpsimd.dma_start(xb[:, HALF:2 * HALF], xv[:, HALF:2 * HALF])

        if prev is not None:
            emit_final(prev)

        sej = spool.tile([P, NJ], FP32, tag="sej", name="sej")
        semf = spool.tile([P, NJ], FP32, tag="semf", name="semf")
        pcs = pacc.tile([1, 2 * C], FP32, tag="pcs")
        sqs = []
        for h in range(2):
            # ---- ACT: spatial-sum accumulator copies (j < NSA) ----
            for jh in range(HJ):
                j = HJ * h + jh
                if j < NSA:
                    dummy = dpool.tile([P, C], BF16, tag="dummy")
                    nc.scalar.activation(dummy[:], xb[:, C * j:C * (j + 1)],
                                         AF.Identity,
                                         accum_out=sej[:, j:j + 1])
            # ---- ACT: squares (feed next iteration's final combine) ----
            for q in range(2):
                sq = sqpool.tile([P, 8 * C], BF16, tag="sq")
                off = (2 * h + q) * 8 * C
                nc.scalar.activation(sq[:], xb[:, off:off + 8 * C], AF.Square)
                sqs.append(sq)
            # ---- PE: channel sum, 8x [128, 512] bf16 matmuls per half ----
            for m8 in range(8):
                m = 8 * h + m8
                nc.tensor.matmul(pcs[:], redcol16[:],
                                 xb[:, 512 * m:512 * (m + 1)],
                                 start=(m == 0), stop=(m == 15),
                                 skip_group_check=True)

        # ---- DVE: channel max over j, flat bf16 fold tree ----
        mh = mpool.tile([P, NJ * C // 2], BF16, tag="mh", name="mh")
        nc.vector.tensor_max(mh[:], xb[:, 0:HALF], xb[:, HALF:2 * HALF])
        w = HALF // 2
        while w >= C:
            nc.vector.tensor_max(mh[:, 0:w], mh[:, 0:w], mh[:, w:2 * w])
            w //= 2

        # ---- DVE: spatial max over c, strided bf16 fold tree ----
        sm = smpool.tile([P, NJ * (C // 2)], BF16, tag="sm", name="sm")
        sm3 = sm[:].rearrange("p (j c) -> p j c", c=C // 2)
        v3 = xb[:].rearrange("p (j c) -> p j c", c=C)
        nc.vector.tensor_max(sm3[:, :, :], v3[:, :, 0:C // 2],
                             v3[:, :, C // 2:C])
        w = C // 4
        while w >= 1:
            out = (semf[:, :].rearrange("p (j o) -> p j o", o=1)
                   if w == 1 else sm3[:, :, 0:w])
            nc.vector.tensor_max(out, sm3[:, :, 0:w], sm3[:, :, w:2 * w])
            w //= 2

        # ---- DVE: spatial sum for j >= NSA, small bf16 add tree ----
        nrem = NJ - NSA
        ss = smpool.tile([P, nrem * (C // 2)], BF16, tag="ss", name="ss")
        ss3 = ss[:].rearrange("p (j c) -> p j c", c=C // 2)
        v8 = xb[:, C * NSA:C * NJ].rearrange("p (j c) -> p j c", c=C)
        nc.vector.tensor_add(ss3[:, :, :], v8[:, :, 0:C // 2],
                             v8[:, :, C // 2:C])
        w = C // 4
        while w >= 1:
            out = (sej[:, NSA:NJ].rearrange("p (j o) -> p j o", o=1)
                   if w == 1 else ss3[:, :, 0:w])
            nc.vector.tensor_add(out, ss3[:, :, 0:w], ss3[:, :, w:2 * w])
            w //= 2

        # ---- channel stats into c-major [128, 4] via PE transposes ----
        avgw = spool.tile([1, 2 * C], FP32, tag="avgw", name="avgw")
        nc.scalar.copy(avgw[:], pcs[0:1, :])
        avg_row = spool.tile([1, C], FP32, tag="avg", name="avg")
        nc.vector.tensor_add(avg_row[:], avgw[0:1, 0:C], avgw[0:1, C:2 * C])
        stats = spool.tile([P, 4], FP32, tag="stats", name="stats")
        # fp32 PSUM bank: po 0:256, ph 256:258, pT 258:260
        wk = pwk.tile([P, C + 4], FP32, tag="wk")
        pT = wk[:, C + 2:C + 4]
        nc.tensor.transpose(pT[:, 0:1], avg_row[0:1, 0:P], ident[0:1, 0:1])
        nc.tensor.transpose(pT[:, 1:2], avg_row[0:1, P:C], ident[0:1, 0:1])
        # bf16 PSUM bank: mT 0:256, sig 256:288
        mtp = pb16.tile([P, 2 * P + NJ], BF16, tag="mtp")
        mT = mtp[:, 0:2 * P]
        nc.tensor.transpose(mT[:, 0:P], mh[:, 0:P], ident16[:, :])
        nc.tensor.transpose(mT[:, P:2 * P], mh[:, P:C], ident16[:, :])
        nc.scalar.copy(stats[:, 0:1], pT[:, 0:1])
        nc.scalar.copy(stats[:, 2:3], pT[:, 1:2])
        nc.vector.tensor_reduce(stats[:, 1:2], mT[:, 0:P],
                                axis=AX.X, op=ALU.max)
        nc.vector.tensor_reduce(stats[:, 3:4], mT[:, P:2 * P],
                                axis=AX.X, op=ALU.max)

        # ---- shared MLP: row HID carries the 2*b2 constant trick ----
        ph = wk[0:HB, C:C + 2]
        nc.tensor.matmul(ph[:], w1[:, 0:HB], stats[:, 0:2],
                         start=True, stop=False, skip_group_check=True)
        nc.tensor.matmul(ph[:], w1[:, HB:2 * HB], stats[:, 2:4],
                         start=False, stop=True, skip_group_check=True)
        hsb = spool.tile([HB, 2], FP32, tag="hsb", name="hsb")
        nc.scalar.activation(hsb[:], ph[:], AF.Relu, bias=b1[:])
        h2 = spool.tile([HB, 1], FP32, tag="h2", name="h2")
        nc.vector.tensor_add(h2[:], hsb[:, 0:1], hsb[:, 1:2])
        h2r = spool.tile([HB, P], FP32, tag="h2r", name="h2r")
        nc.scalar.mul(h2r[:], ones[0:HB, :], h2[:])
        po = wk[:, 0:C]
        nc.tensor.matmul(po[:], h2r[:], w2b[:], start=True, stop=True,
                         skip_group_check=True)
        att = spool.tile([P, C], BF16, tag="att", name="att")
        nc.scalar.activation(att[:], po[:], AF.Sigmoid)

        # ---- 7-tap conv in transposed [j, p] space (PE Toeplitz) ----
        # fp32 PSUM bank: sjT 0:128, smT 128:256, pcv 256:384
        pct = pcnv.tile([NJ, 3 * P], FP32, tag="pct")
        nc.tensor.transpose(pct[:, 0:P], sej[:], ident[:, :])
        nc.tensor.transpose(pct[:, P:2 * P], semf[:], ident[:, :])
        sjS = spool.tile([NJ, P], FP32, tag="sjS", name="sjS")
        nc.scalar.copy(sjS[:], pct[:, 0:P])
        smS = spool.tile([NJ, P], FP32, tag="smS", name="smS")
        nc.scalar.copy(smS[:], pct[:, P:2 * P])
        pcv = pct[:, 2 * P:3 * P]
        nc.tensor.matmul(pcv[:, :], tj[:, 0:NJ], sjS[:, :],
                         start=True, stop=False, skip_group_check=True)
        nc.tensor.matmul(pcv[:, 1:P], tj[:, NJ:2 * NJ], sjS[:, 0:P - 1],
                         start=False, stop=False, skip_group_check=True)
        nc.tensor.matmul(pcv[:, 0:P - 1], tj[:, 2 * NJ:3 * NJ], sjS[:, 1:P],
                         start=False, stop=False, skip_group_check=True)
        nc.tensor.matmul(pcv[:, :], tj[:, 3 * NJ:4 * NJ], smS[:, :],
                         start=False, stop=False, skip_group_check=True)
        nc.tensor.matmul(pcv[:, 1:P], tj[:, 4 * NJ:5 * NJ], smS[:, 0:P - 1],
                         start=False, stop=False, skip_group_check=True)
        nc.tensor.matmul(pcv[:, 0:P - 1], tj[:, 5 * NJ:6 * NJ], smS[:, 1:P],
                         start=False, stop=True, skip_group_check=True)
        sigT = spool.tile([NJ, P], BF16, tag="sigT", name="sigT")
        nc.scalar.activation(sigT[:], pcv[:], AF.Sigmoid)
        psig = mtp[:, 2 * P:2 * P + NJ]
        nc.tensor.transpose(psig[:], sigT[:], ident16[0:NJ, 0:NJ])
        sig_sb = spool.tile([P, NJ], BF16, tag="sig_sb", name="sig_sb")
        nc.scalar.copy(sig_sb[:], psig[:])

        prev = (att, sig_sb, sqs, b)

    emit_final(prev)


def _build_nc(reps=1):
    nc = bacc.Bacc("TRN2", target_bir_lowering=False, debug=False,
                   enable_asserts=False, num_devices=N_CORES)
    x_d = nc.dram_tensor("xin", [NB, L, C], FP32, kind="ExternalInput").ap()
    w1_d = nc.dram_tensor("w1sb", [P, 2 * HB], FP32, kind="ExternalInput").ap()
    b1_d = nc.dram_tensor("b1col", [HB, 1], FP32, kind="ExternalInput").ap()
    w2b_d = nc.dram_tensor("w2b", [HB, C], FP32, kind="ExternalInput").ap()
    id_d = nc.dram_tensor("ident", [P, P], FP32, kind="ExternalInput").ap()
    id16_d = nc.dram_tensor("ident16", [P, P], BF16, kind="ExternalInput").ap()
    ones_d = nc.dram_tensor("ones", [P, P], FP32, kind="ExternalInput").ap()
    rc16_d = nc.dram_tensor("redcol16", [P, 1], BF16, kind="ExternalInput").ap()
    tj_d = nc.dram_tensor("tjconv", [NJ, 6 * NJ], FP32, kind="ExternalInput").ap()
    out_d = nc.dram_tensor("out", [NB, L, C], FP32, kind="ExternalOutput").ap()

    with tile.TileContext(nc) as tc:
        with ExitStack() as ctx:
            _build_body(ctx, tc, out_d, x_d, w1_d, b1_d, w2b_d, id_d,
                        id16_d, ones_d, rc16_d, tj_d, reps=reps)
    nc.compile()
    return nc


def get_nc(reps=1):
    key = f"nc{reps}"
    if key not in _CACHE:
        _CACHE[key] = _build_nc(reps=reps)
    return _CACHE[key]


def _prep_inputs(W1, b1, W2, b2, conv_w):
    """Host-side parameter preprocessing (shared across cores)."""
    W1 = np.asarray(W1, np.float32)
    W2 = np.asarray(W2, np.float32)
    b1 = np.asarray(b1, np.float32)
    b2 = np.asarray(b2, np.float32)
    conv_w = np.asarray(conv_w, np.float32)

    w1sb = np.zeros((P, 2 * HB), np.float32)
    for h in range(2):
        w1sb[:, HB * h:HB * h + HID] = W1[P * h:P * (h + 1), :]
    w2b = np.concatenate([W2, b2[None, :]], axis=0).astype(np.float32)
    b1col = np.concatenate([b1, [1.0]]).astype(np.float32).reshape(HB, 1)

    # Transposed-space conv Toeplitz lhsTs [j', j]; the avg tap folds in the
    # 1/C spatial-mean scale (device computes raw channel sums).
    wa = (conv_w[:, 0, 0] / C).astype(np.float32)
    wm = conv_w[:, 1, 0].astype(np.float32)
    tj = np.zeros((NJ, 6 * NJ), np.float32)
    for jp in range(NJ):
        for j in range(NJ):
            k = jp - j + 3          # main band
            if 0 <= k < 7:
                tj[jp, j] = wa[k]
                tj[jp, 3 * NJ + j] = wm[k]
            k = jp - j - 29         # prev-partition corner
            if 0 <= k < 7 and jp >= 29 and j <= 2:
                tj[jp, NJ + j] = wa[k]
                tj[jp, 4 * NJ + j] = wm[k]
            k = jp + 35 - j         # next-partition corner
            if 0 <= k < 7 and jp <= 2 and j >= 29:
                tj[jp, 2 * NJ + j] = wa[k]
                tj[jp, 5 * NJ + j] = wm[k]

    import ml_dtypes
    bf16 = ml_dtypes.bfloat16
    return {
        "w1sb": w1sb,
        "b1col": np.ascontiguousarray(b1col),
        "w2b": w2b,
        "ident": np.eye(P, dtype=np.float32),
        "ident16": np.eye(P, dtype=bf16),
        "ones": np.ones((P, P), np.float32),
        "redcol16": np.full((P, 1), 1.0 / L, bf16),
        "tjconv": tj,
    }


def kernel(x, W1, b1, W2, b2, conv_w):
    nc = get_nc()
    x = np.asarray(x, np.float32)
    params = _prep_inputs(W1, b1, W2, b2, conv_w)
    in_maps = []
    for c in range(N_CORES):
        m = dict(params)
        m["xin"] = np.ascontiguousarray(x[NB * c:NB * (c + 1)])
        in_maps.append(m)
    _CACHE["last_in_maps"] = in_maps
    res = run_bass_kernel_spmd(nc, in_maps, list(range(N_CORES)))
    _CACHE["last_results"] = res
    return np.concatenate([res.results[c]["out"] for c in range(N_CORES)],
                          axis=0)


def _pjrt_exec(nc, in_maps, n_warm=2, n_time=8):
    """Build a sharded jit for nc, run it, return (best_wall_s, result)."""
    import time
    import jax
    import concourse.mybir as mybir_
    from concourse.bass2jax import (_bass_exec_p, install_neuronx_cc_hook,
                                    partition_id_tensor)
    from jax.experimental.shard_map import shard_map
    from jax.sharding import Mesh, PartitionSpec

    install_neuronx_cc_hook()
    partition_name = (nc.partition_id_tensor.name
                      if nc.partition_id_tensor else None)
    in_names, out_names, out_avals = [], [], []
    for alloc in nc.m.functions[0].allocations:
        if not isinstance(alloc, mybir_.MemoryLocationSet):
            continue
        name = alloc.memorylocations[0].name
        if alloc.kind == "ExternalInput":
            if name != partition_name:
                in_names.append(name)
        elif alloc.kind == "ExternalOutput":
            out_names.append(name)
            out_avals.append(jax.core.ShapedArray(
                tuple(alloc.tensor_shape), mybir_.dt.np(alloc.dtype)))
    n_params = len(in_names)
    all_in_names = list(in_names) + list(out_names)
    if partition_name is not None:
        all_in_names.append(partition_name)

    def _body(*args):
        operands = list(args)
        if partition_name is not None:
            operands.append(partition_id_tensor())
        return tuple(_bass_exec_p.bind(
            *operands,
            out_avals=tuple(out_avals),
            in_names=tuple(all_in_names),
            out_names=tuple(out_names),
            lowering_input_output_aliases=(),
            sim_require_finite=True,
            sim_require_nnan=True,
            nc=nc,
        ))

    devices = jax.devices()[:N_CORES]
    mesh = Mesh(np.asarray(devices), ("core",))
    nin = n_params + len(out_names)
    sharding = jax.sharding.NamedSharding(mesh, PartitionSpec("core"))
    fn = jax.jit(shard_map(
        _body, mesh=mesh,
        in_specs=(PartitionSpec("core"),) * nin,
        out_specs=(PartitionSpec("core"),) * len(out_names),
        check_rep=False))
    dev_args = [
        jax.device_put(np.concatenate(
            [np.asarray(in_maps[c][nm]) for c in range(N_CORES)], axis=0),
            sharding)
        for nm in in_names
    ]
    for av in out_avals:
        z = np.zeros((N_CORES * av.shape[0], *av.shape[1:]), av.dtype)
        dev_args.append(jax.device_put(z, sharding))

    for _ in range(n_warm):
        out = fn(*dev_args)
        jax.block_until_ready(out)
    best = float("inf")
    for _ in range(n_time):
        t0 = time.perf_counter()
        out = fn(*dev_args)
        jax.block_until_ready(out)
        best = min(best, time.perf_counter() - t0)
    result = np.asarray(out[0]).reshape(N_CORES * NB, L, C)
    return best, result


def bench_repeat(reps=8, n_time=10, in_maps=None):
    """Isolate device exec time: time a module doing the work `reps` times
    in-kernel vs once; slope = steady-state HW time per execution."""
    if in_maps is None:
        in_maps = _CACHE["last_in_maps"]
    t1, _ = _pjrt_exec(get_nc(1), in_maps, n_time=n_time)
    tr, result = _pjrt_exec(get_nc(reps), in_maps, n_time=n_time)
    per_exec_ns = (tr - t1) / (reps - 1) * 1e9
    return per_exec_ns, result, t1 * 1e9, tr * 1e9


def bench(n_iters=30, in_maps=None):
    """Time back-to-back NEFF executions with device-resident inputs."""
    import time
    import jax
    import concourse.mybir as mybir_
    from concourse.bass2jax import (_bass_exec_p, install_neuronx_cc_hook,
                                    partition_id_tensor)
    from jax.experimental.shard_map import shard_map
    from jax.sharding import Mesh, PartitionSpec

    nc = get_nc()
    if in_maps is None:
        in_maps = _CACHE["last_in_maps"]
    install_neuronx_cc_hook()

    partition_name = (nc.partition_id_tensor.name
                      if nc.partition_id_tensor else None)
    in_names, out_names, out_avals, zero_outs = [], [], [], []
    for alloc in nc.m.functions[0].allocations:
        if not isinstance(alloc, mybir_.MemoryLocationSet):
            continue
        name = alloc.memorylocations[0].name
        if alloc.kind == "ExternalInput":
            if name != partition_name:
                in_names.append(name)
        elif alloc.kind == "ExternalOutput":
            shape = tuple(alloc.tensor_shape)
            dtype = mybir_.dt.np(alloc.dtype)
            out_names.append(name)
            out_avals.append(jax.core.ShapedArray(shape, dtype))
            zero_outs.append(np.zeros(shape, dtype))
    n_params = len(in_names)
    all_in_names = list(in_names) + list(out_names)
    if partition_name is not None:
        all_in_names.append(partition_name)

    def _body(*args):
        operands = list(args)
        if partition_name is not None:
            operands.append(partition_id_tensor())
        return tuple(_bass_exec_p.bind(
            *operands,
            out_avals=tuple(out_avals),
            in_names=tuple(all_in_names),
            out_names=tuple(out_names),
            lowering_input_output_aliases=(),
            sim_require_finite=True,
            sim_require_nnan=True,
            nc=nc,
        ))

    devices = jax.devices()[:N_CORES]
    mesh = Mesh(np.asarray(devices), ("core",))
    nin = n_params + len(out_names)
    sharded = jax.jit(shard_map(
        _body, mesh=mesh,
        in_specs=(PartitionSpec("core"),) * nin,
        out_specs=(PartitionSpec("core"),) * len(out_names),
        check_rep=False))

    concat_in = [
        np.concatenate([np.asarray(in_maps[c][nm]) for c in range(N_CORES)],
                       axis=0)
        for nm in in_names
    ]
    concat_zeros = [
        np.zeros((N_CORES * z.shape[0], *z.shape[1:]), z.dtype)
        for z in zero_outs
    ]
    sharding = jax.sharding.NamedSharding(mesh, PartitionSpec("core"))
    dev_args = [jax.device_put(a, sharding) for a in concat_in + concat_zeros]

    out = sharded(*dev_args)
    jax.block_until_ready(out)
    t0 = time.perf_counter()
    for _ in range(n_iters):
        out = sharded(*dev_args)
    jax.block_until_ready(out)
    t1 = time.perf_counter()
    per_iter_ns = (t1 - t0) / n_iters * 1e9
    result = np.asarray(out[0]).reshape(N_CORES * NB, L, C)
    return per_iter_ns, result


# revision 11
# speedup vs baseline: 1.6487x; 1.0501x over previous
"""CBAM kernel for Trainium2, 8-way batch-parallel SPMD.

Computes out = x^2 * (att_c[b,c] + sigmoid(conv(spatial_stats))[b,l]) where
att_c = sigmoid(mlp(mean_L x) + mlp(max_L x)), matching the CBAM reference.

Layout per core: 4 batches; each batch x[4096, 256] lives in SBUF as one
[128, 8192] bf16 tensor (partition p holds DRAM rows [32p, 32p+32), i.e.
32 KiB contiguous fp32 DRAM per partition; the fp32->bf16 cast happens for
free inside the SWDGE load DMA). l = 32*p + j, free col = 256*j + c.

Engine split per batch (~23 us DMA window):
  DVE  : channel-max + spatial-max bf16 fold trees (2x perf mode), 8-j
         spatial-sum tree, the 32 final (att+sig)*x^2 STTs (deferred one
         batch so they fill the next load window), small reduces
  ACT  : 24-j spatial-sum accumulator copies, squares (bf16), sigmoids,
         PSUM->SBUF copies
  PE   : channel-sum (16x [128,512] bf16 matmuls), stats transposes, MLP,
         7-tap conv as banded-Toeplitz matmuls in transposed [j, p] space
         (corner taps = column-shifted rhs, no halo exchange needed)
  POOL : SWDGE DMA queue only - cast loads, cast stores (bf16 -> fp32)
"""

import numpy as np
from contextlib import ExitStack

import concourse.bacc as bacc
import concourse.bass as bass
import concourse.tile as tile
import concourse.mybir as mybir
from concourse.bass_utils import run_bass_kernel_spmd

AF = mybir.ActivationFunctionType
ALU = mybir.AluOpType
AX = mybir.AxisListType
FP32 = mybir.dt.float32
BF16 = mybir.dt.bfloat16

N_CORES = 8
B_FULL = 32
NB = B_FULL // N_CORES  # batches per core = 4
L = 4096
C = 256
HID = 16
HB = HID + 1
P = 128
NJ = L // P  # 32 j-blocks (rows per partition)
HJ = NJ // 2  # 16 j-blocks per half
HALF = HJ * C  # 4096 free columns per half

_CACHE: dict = {}


def _build_body(ctx: ExitStack, tc, out_d, x_d, w1_d, b1_d, w2b_d, id_d,
                id16_d, ones_d, rc16_d, tj_d, reps=1):
    nc = tc.nc

    const = ctx.enter_context(tc.tile_pool(name="const", bufs=1))
    bpool = ctx.enter_context(tc.tile_pool(name="xb16", bufs=3))
    opool = ctx.enter_context(tc.tile_pool(name="outt", bufs=2))
    sqpool = ctx.enter_context(tc.tile_pool(name="sq", bufs=8))
    mpool = ctx.enter_context(tc.tile_pool(name="maxtree", bufs=1))
    smpool = ctx.enter_context(tc.tile_pool(name="spattree", bufs=1))
    spool = ctx.enter_context(tc.tile_pool(name="stats", bufs=2))
    dpool = ctx.enter_context(tc.tile_pool(name="dummy", bufs=2))
    pacc = ctx.enter_context(tc.tile_pool(name="pacc", bufs=2, space="PSUM"))
    pwk = ctx.enter_context(tc.tile_pool(name="pwk", bufs=2, space="PSUM"))
    pcnv = ctx.enter_context(tc.tile_pool(name="pcnv", bufs=2, space="PSUM"))
    pb16 = ctx.enter_context(tc.tile_pool(name="pb16", bufs=2, space="PSUM"))

    w1 = const.tile([P, 2 * HB], FP32)
    nc.sync.dma_start(w1[:], w1_d[:])
    b1 = const.tile([HB, 1], FP32)
    nc.sync.dma_start(b1[:], b1_d[:])
    w2b = const.tile([HB, C], FP32)
    nc.sync.dma_start(w2b[:], w2b_d[:])
    ident = const.tile([P, P], FP32)
    nc.sync.dma_start(ident[:], id_d[:])
    ident16 = const.tile([P, P], BF16)
    nc.sync.dma_start(ident16[:], id16_d[:])
    ones = const.tile([P, P], FP32)
    nc.sync.dma_start(ones[:], ones_d[:])
    redcol16 = const.tile([P, 1], BF16)
    nc.sync.dma_start(redcol16[:], rc16_d[:])
    tj = const.tile([NJ, 6 * NJ], FP32)
    nc.sync.dma_start(tj[:], tj_d[:])

    NSA = 20  # j-blocks whose spatial sum rides ACT accumulator copies

    def emit_final(prev):
        """Final combine (att + sig) * x^2 for the previous batch + stores.

        Emitted at the top of the next iteration so the 32 DVE STTs fill
        the load window of the current batch, and the stores land on the
        SWDGE queue right behind the current batch's loads."""
        att, psig, sqs, pb = prev
        ot = opool.tile([P, NJ * C], FP32, tag="ot", name="ot")
        for s in range(4):
            for j8 in range(8):
                j = 8 * s + j8
                nc.vector.scalar_tensor_tensor(ot[:, C * j:C * (j + 1)],
                                               att[:], psig[:, j:j + 1],
                                               sqs[s][:, C * j8:C * (j8 + 1)],
                                               op0=ALU.add, op1=ALU.mult)
        ov = out_d[pb, :, :].rearrange("(p q) c -> p (q c)", p=P)
        nc.sync.dma_start(ov[:, 0:HALF], ot[:, 0:HALF])
        nc.sync.dma_start(ov[:, HALF:2 * HALF], ot[:, HALF:2 * HALF])

    prev = None
    for b in [b for _ in range(reps) for b in range(NB)]:
        xb = bpool.tile([P, NJ * C], BF16, tag="xb", name="xb")
        xv = x_d[b, :, :].rearrange("(p q) c -> p (q c)", p=P)
        QW = HALF // 2
        for q4 in range(4):
            nc.gpsimd.dma_start(xb[:, QW * q4:QW * (q4 + 1)],
                                xv[:, QW * q4:QW * (q4 + 1)])

        if prev is not None:
            emit_final(prev)

        sej = spool.tile([P, NJ], FP32, tag="sej", name="sej")
        semf = spool.tile([P, NJ], FP32, tag="semf", name="semf")
        pcs = pacc.tile([1, 2 * C], FP32, tag="pcs")
        sqs = []
        for h in range(2):
            # ---- ACT: spatial-sum accumulator copies (j < NSA) ----
            for jh in range(HJ):
                j = HJ * h + jh
                if j < NSA:
                    dummy = dpool.tile([P, C], BF16, tag="dummy")
                    nc.scalar.activation(dummy[:], xb[:, C * j:C * (j + 1)],
                                         AF.Identity,
                                         accum_out=sej[:, j:j + 1])
            # ---- ACT: squares (feed next iteration's final combine) ----
            for q in range(2):
                sq = sqpool.tile([P, 8 * C], BF16, tag="sq")
                off = (2 * h + q) * 8 * C
                nc.scalar.activation(sq[:], xb[:, off:off + 8 * C], AF.Square)
                sqs.append(sq)
            # ---- PE: channel sum, 8x [128, 512] bf16 matmuls per half ----
            for m8 in range(8):
                m = 8 * h + m8
                nc.tensor.matmul(pcs[:], redcol16[:],
                                 xb[:, 512 * m:512 * (m + 1)],
                                 start=(m == 0), stop=(m == 15),
                                 skip_group_check=True)

        # ---- DVE: channel max over j, flat bf16 fold tree ----
        mh = mpool.tile([P, NJ * C // 2], BF16, tag="mh", name="mh")
        nc.vector.tensor_max(mh[:], xb[:, 0:HALF], xb[:, HALF:2 * HALF])
        w = HALF // 2
        while w >= C:
            nc.vector.tensor_max(mh[:, 0:w], mh[:, 0:w], mh[:, w:2 * w])
            w //= 2

        # ---- DVE: spatial max over c, strided bf16 fold tree ----
        sm = smpool.tile([P, NJ * (C // 2)], BF16, tag="sm", name="sm")
        sm3 = sm[:].rearrange("p (j c) -> p j c", c=C // 2)
        v3 = xb[:].rearrange("p (j c) -> p j c", c=C)
        nc.vector.tensor_max(sm3[:, :, :], v3[:, :, 0:C // 2],
                             v3[:, :, C // 2:C])
        w = C // 4
        while w >= 1:
            out = (semf[:, :].rearrange("p (j o) -> p j o", o=1)
                   if w == 1 else sm3[:, :, 0:w])
            nc.vector.tensor_max(out, sm3[:, :, 0:w], sm3[:, :, w:2 * w])
            w //= 2

        # ---- DVE: spatial sum for j >= NSA, small bf16 add tree ----
        nrem = NJ - NSA
        ss = smpool.tile([P, nrem * (C // 2)], BF16, tag="ss", name="ss")
        ss3 = ss[:].rearrange("p (j c) -> p j c", c=C // 2)
        v8 = xb[:, C * NSA:C * NJ].rearrange("p (j c) -> p j c", c=C)
        nc.vector.tensor_add(ss3[:, :, :], v8[:, :, 0:C // 2],
                             v8[:, :, C // 2:C])
        w = C // 4
        while w >= 1:
            out = (sej[:, NSA:NJ].rearrange("p (j o) -> p j o", o=1)
                   if w == 1 else ss3[:, :, 0:w])
            nc.vector.tensor_add(out, ss3[:, :, 0:w], ss3[:, :, w:2 * w])
            w //= 2

        # ---- channel stats into c-major [128, 4] via PE transposes ----
        avgw = spool.tile([1, 2 * C], FP32, tag="avgw", name="avgw")
        nc.scalar.copy(avgw[:], pcs[0:1, :])
        avg_row = spool.tile([1, C], FP32, tag="avg", name="avg")
        nc.vector.tensor_add(avg_row[:], avgw[0:1, 0:C], avgw[0:1, C:2 * C])
        stats = spool.tile([P, 4], FP32, tag="stats", name="stats")
        # fp32 PSUM bank: po 0:256, ph 256:258, pT 258:260
        wk = pwk.tile([P, C + 4], FP32, tag="wk")
        pT = wk[:, C + 2:C + 4]
        nc.tensor.transpose(pT[:, 0:1], avg_row[0:1, 0:P], ident[0:1, 0:1])
        nc.tensor.transpose(pT[:, 1:2], avg_row[0:1, P:C], ident[0:1, 0:1])
        # bf16 PSUM bank: mT 0:256, sig 256:288
        mtp = pb16.tile([P, 2 * P + NJ], BF16, tag="mtp")
        mT = mtp[:, 0:2 * P]
        nc.tensor.transpose(mT[:, 0:P], mh[:, 0:P], ident16[:, :])
        nc.tensor.transpose(mT[:, P:2 * P], mh[:, P:C], ident16[:, :])
        nc.scalar.copy(stats[:, 0:1], pT[:, 0:1])
        nc.scalar.copy(stats[:, 2:3], pT[:, 1:2])
        nc.vector.tensor_reduce(stats[:, 1:2], mT[:, 0:P],
                                axis=AX.X, op=ALU.max)
        nc.vector.tensor_reduce(stats[:, 3:4], mT[:, P:2 * P],
                                axis=AX.X, op=ALU.max)

        # ---- shared MLP: row HID carries the 2*b2 constant trick ----
        ph = wk[0:HB, C:C + 2]
        nc.tensor.matmul(ph[:], w1[:, 0:HB], stats[:, 0:2],
                         start=True, stop=False, skip_group_check=True)
        nc.tensor.matmul(ph[:], w1[:, HB:2 * HB], stats[:, 2:4],
                         start=False, stop=True, skip_group_check=True)
        hsb = spool.tile([HB, 2], FP32, tag="hsb", name="hsb")
        nc.scalar.activation(hsb[:], ph[:], AF.Relu, bias=b1[:])
        h2 = spool.tile([HB, 1], FP32, tag="h2", name="h2")
        nc.vector.tensor_add(h2[:], hsb[:, 0:1], hsb[:, 1:2])
        h2r = spool.tile([HB, P], FP32, tag="h2r", name="h2r")
        nc.scalar.mul(h2r[:], ones[0:HB, :], h2[:])
        po = wk[:, 0:C]
        nc.tensor.matmul(po[:], h2r[:], w2b[:], start=True, stop=True,
                         skip_group_check=True)
        att = spool.tile([P, C], BF16, tag="att", name="att")
        nc.scalar.activation(att[:], po[:], AF.Sigmoid)

        # ---- 7-tap conv in transposed [j, p] space (PE Toeplitz) ----
        # fp32 PSUM bank: sjT 0:128, smT 128:256, pcv 256:384
        pct = pcnv.tile([NJ, 3 * P], FP32, tag="pct")
        nc.tensor.transpose(pct[:, 0:P], sej[:], ident[:, :])
        nc.tensor.transpose(pct[:, P:2 * P], semf[:], ident[:, :])
        sjS = spool.tile([NJ, P], FP32, tag="sjS", name="sjS")
        nc.scalar.copy(sjS[:], pct[:, 0:P])
        smS = spool.tile([NJ, P], FP32, tag="smS", name="smS")
        nc.scalar.copy(smS[:], pct[:, P:2 * P])
        pcv = pct[:, 2 * P:3 * P]
        nc.tensor.matmul(pcv[:, :], tj[:, 0:NJ], sjS[:, :],
                         start=True, stop=False, skip_group_check=True)
        nc.tensor.matmul(pcv[:, 1:P], tj[:, NJ:2 * NJ], sjS[:, 0:P - 1],
                         start=False, stop=False, skip_group_check=True)
        nc.tensor.matmul(pcv[:, 0:P - 1], tj[:, 2 * NJ:3 * NJ], sjS[:, 1:P],
                         start=False, stop=False, skip_group_check=True)
        nc.tensor.matmul(pcv[:, :], tj[:, 3 * NJ:4 * NJ], smS[:, :],
                         start=False, stop=False, skip_group_check=True)
        nc.tensor.matmul(pcv[:, 1:P], tj[:, 4 * NJ:5 * NJ], smS[:, 0:P - 1],
                         start=False, stop=False, skip_group_check=True)
        nc.tensor.matmul(pcv[:, 0:P - 1], tj[:, 5 * NJ:6 * NJ], smS[:, 1:P],
                         start=False, stop=True, skip_group_check=True)
        sigT = spool.tile([NJ, P], BF16, tag="sigT", name="sigT")
        nc.scalar.activation(sigT[:], pcv[:], AF.Sigmoid)
        psig = mtp[:, 2 * P:2 * P + NJ]
        nc.tensor.transpose(psig[:], sigT[:], ident16[0:NJ, 0:NJ])
        sig_sb = spool.tile([P, NJ], BF16, tag="sig_sb", name="sig_sb")
        nc.scalar.copy(sig_sb[:], psig[:])

        prev = (att, sig_sb, sqs, b)

    emit_final(prev)


def _build_nc(reps=1):
    nc = bacc.Bacc("TRN2", target_bir_lowering=False, debug=False,
                   enable_asserts=False, num_devices=N_CORES)
    x_d = nc.dram_tensor("xin", [NB, L, C], FP32, kind="ExternalInput").ap()
    w1_d = nc.dram_tensor("w1sb", [P, 2 * HB], FP32, kind="ExternalInput").ap()
    b1_d = nc.dram_tensor("b1col", [HB, 1], FP32, kind="ExternalInput").ap()
    w2b_d = nc.dram_tensor("w2b", [HB, C], FP32, kind="ExternalInput").ap()
    id_d = nc.dram_tensor("ident", [P, P], FP32, kind="ExternalInput").ap()
    id16_d = nc.dram_tensor("ident16", [P, P], BF16, kind="ExternalInput").ap()
    ones_d = nc.dram_tensor("ones", [P, P], FP32, kind="ExternalInput").ap()
    rc16_d = nc.dram_tensor("redcol16", [P, 1], BF16, kind="ExternalInput").ap()
    tj_d = nc.dram_tensor("tjconv", [NJ, 6 * NJ], FP32, kind="ExternalInput").ap()
    out_d = nc.dram_tensor("out", [NB, L, C], FP32, kind="ExternalOutput").ap()

    with tile.TileContext(nc) as tc:
        with ExitStack() as ctx:
            _build_body(ctx, tc, out_d, x_d, w1_d, b1_d, w2b_d, id_d,
                        id16_d, ones_d, rc16_d, tj_d, reps=reps)
    nc.compile()
    return nc


def get_nc(reps=1):
    key = f"nc{reps}"
    if key not in _CACHE:
        _CACHE[key] = _build_nc(reps=reps)
    return _CACHE[key]


def _prep_inputs(W1, b1, W2, b2, conv_w):
    """Host-side parameter preprocessing (shared across cores)."""
    W1 = np.asarray(W1, np.float32)
    W2 = np.asarray(W2, np.float32)
    b1 = np.asarray(b1, np.float32)
    b2 = np.asarray(b2, np.float32)
    conv_w = np.asarray(conv_w, np.float32)

    w1sb = np.zeros((P, 2 * HB), np.float32)
    for h in range(2):
        w1sb[:, HB * h:HB * h + HID] = W1[P * h:P * (h + 1), :]
    w2b = np.concatenate([W2, b2[None, :]], axis=0).astype(np.float32)
    b1col = np.concatenate([b1, [1.0]]).astype(np.float32).reshape(HB, 1)

    # Transposed-space conv Toeplitz lhsTs [j', j]; the avg tap folds in the
    # 1/C spatial-mean scale (device computes raw channel sums).
    wa = (conv_w[:, 0, 0] / C).astype(np.float32)
    wm = conv_w[:, 1, 0].astype(np.float32)
    tj = np.zeros((NJ, 6 * NJ), np.float32)
    for jp in range(NJ):
        for j in range(NJ):
            k = jp - j + 3          # main band
            if 0 <= k < 7:
                tj[jp, j] = wa[k]
                tj[jp, 3 * NJ + j] = wm[k]
            k = jp - j - 29         # prev-partition corner
            if 0 <= k < 7 and jp >= 29 and j <= 2:
                tj[jp, NJ + j] = wa[k]
                tj[jp, 4 * NJ + j] = wm[k]
            k = jp + 35 - j         # next-partition corner
            if 0 <= k < 7 and jp <= 2 and j >= 29:
                tj[jp, 2 * NJ + j] = wa[k]
                tj[jp, 5 * NJ + j] = wm[k]

    import ml_dtypes
    bf16 = ml_dtypes.bfloat16
    return {
        "w1sb": w1sb,
        "b1col": np.ascontiguousarray(b1col),
        "w2b": w2b,
        "ident": np.eye(P, dtype=np.float32),
        "ident16": np.eye(P, dtype=bf16),
        "ones": np.ones((P, P), np.float32),
        "redcol16": np.full((P, 1), 1.0 / L, bf16),
        "tjconv": tj,
    }


def kernel(x, W1, b1, W2, b2, conv_w):
    nc = get_nc()
    x = np.asarray(x, np.float32)
    params = _prep_inputs(W1, b1, W2, b2, conv_w)
    in_maps = []
    for c in range(N_CORES):
        m = dict(params)
        m["xin"] = np.ascontiguousarray(x[NB * c:NB * (c + 1)])
        in_maps.append(m)
    _CACHE["last_in_maps"] = in_maps
    res = run_bass_kernel_spmd(nc, in_maps, list(range(N_CORES)))
    _CACHE["last_results"] = res
    return np.concatenate([res.results[c]["out"] for c in range(N_CORES)],
                          axis=0)


def _pjrt_exec(nc, in_maps, n_warm=2, n_time=8):
    """Build a sharded jit for nc, run it, return (best_wall_s, result)."""
    import time
    import jax
    import concourse.mybir as mybir_
    from concourse.bass2jax import (_bass_exec_p, install_neuronx_cc_hook,
                                    partition_id_tensor)
    from jax.experimental.shard_map import shard_map
    from jax.sharding import Mesh, PartitionSpec

    install_neuronx_cc_hook()
    partition_name = (nc.partition_id_tensor.name
                      if nc.partition_id_tensor else None)
    in_names, out_names, out_avals = [], [], []
    for alloc in nc.m.functions[0].allocations:
        if not isinstance(alloc, mybir_.MemoryLocationSet):
            continue
        name = alloc.memorylocations[0].name
        if alloc.kind == "ExternalInput":
            if name != partition_name:
                in_names.append(name)
        elif alloc.kind == "ExternalOutput":
            out_names.append(name)
            out_avals.append(jax.core.ShapedArray(
                tuple(alloc.tensor_shape), mybir_.dt.np(alloc.dtype)))
    n_params = len(in_names)
    all_in_names = list(in_names) + list(out_names)
    if partition_name is not None:
        all_in_names.append(partition_name)

    def _body(*args):
        operands = list(args)
        if partition_name is not None:
            operands.append(partition_id_tensor())
        return tuple(_bass_exec_p.bind(
            *operands,
            out_avals=tuple(out_avals),
            in_names=tuple(all_in_names),
            out_names=tuple(out_names),
            lowering_input_output_aliases=(),
            sim_require_finite=True,
            sim_require_nnan=True,
            nc=nc,
        ))

    devices = jax.devices()[:N_CORES]
    mesh = Mesh(np.asarray(devices), ("core",))
    nin = n_params + len(out_names)
    sharding = jax.sharding.NamedSharding(mesh, PartitionSpec("core"))
    fn = jax.jit(shard_map(
        _body, mesh=mesh,
        in_specs=(PartitionSpec("core"),) * nin,
        out_specs=(PartitionSpec("core"),) * len(out_names),
        check_rep=False))
    dev_args = [
        jax.device_put(np.concatenate(
            [np.asarray(in_maps[c][nm]) for c in range(N_CORES)], axis=0),
            sharding)
        for nm in in_names
    ]
    for av in out_avals:
        z = np.zeros((N_CORES * av.shape[0], *av.shape[1:]), av.dtype)
        dev_args.append(jax.device_put(z, sharding))

    for _ in range(n_warm):
        out = fn(*dev_args)
        jax.block_until_ready(out)
    best = float("inf")
    for _ in range(n_time):
        t0 = time.perf_counter()
        out = fn(*dev_args)
        jax.block_until_ready(out)
        best = min(best, time.perf_counter() - t0)
    result = np.asarray(out[0]).reshape(N_CORES * NB, L, C)
    return best, result


def bench_repeat(reps=8, n_time=10, in_maps=None):
    """Isolate device exec time: time a module doing the work `reps` times
    in-kernel vs once; slope = steady-state HW time per execution."""
    if in_maps is None:
        in_maps = _CACHE["last_in_maps"]
    t1, _ = _pjrt_exec(get_nc(1), in_maps, n_time=n_time)
    tr, result = _pjrt_exec(get_nc(reps), in_maps, n_time=n_time)
    per_exec_ns = (tr - t1) / (reps - 1) * 1e9
    return per_exec_ns, result, t1 * 1e9, tr * 1e9


def bench(n_iters=30, in_maps=None):
    """Time back-to-back NEFF executions with device-resident inputs."""
    import time
    import jax
    import concourse.mybir as mybir_
    from concourse.bass2jax import (_bass_exec_p, install_neuronx_cc_hook,
                                    partition_id_tensor)
    from jax.experimental.shard_map import shard_map
    from jax.sharding import Mesh, PartitionSpec

    nc = get_nc()
    if in_maps is None:
        in_maps = _CACHE["last_in_maps"]
    install_neuronx_cc_hook()

    partition_name = (nc.partition_id_tensor.name
                      if nc.partition_id_tensor else None)
    in_names, out_names, out_avals, zero_outs = [], [], [], []
    for alloc in nc.m.functions[0].allocations:
        if not isinstance(alloc, mybir_.MemoryLocationSet):
            continue
        name = alloc.memorylocations[0].name
        if alloc.kind == "ExternalInput":
            if name != partition_name:
                in_names.append(name)
        elif alloc.kind == "ExternalOutput":
            shape = tuple(alloc.tensor_shape)
            dtype = mybir_.dt.np(alloc.dtype)
            out_names.append(name)
            out_avals.append(jax.core.ShapedArray(shape, dtype))
            zero_outs.append(np.zeros(shape, dtype))
    n_params = len(in_names)
    all_in_names = list(in_names) + list(out_names)
    if partition_name is not None:
        all_in_names.append(partition_name)

    def _body(*args):
        operands = list(args)
        if partition_name is not None:
            operands.append(partition_id_tensor())
        return tuple(_bass_exec_p.bind(
            *operands,
            out_avals=tuple(out_avals),
            in_names=tuple(all_in_names),
            out_names=tuple(out_names),
            lowering_input_output_aliases=(),
            sim_require_finite=True,
            sim_require_nnan=True,
            nc=nc,
        ))

    devices = jax.devices()[:N_CORES]
    mesh = Mesh(np.asarray(devices), ("core",))
    nin = n_params + len(out_names)
    sharded = jax.jit(shard_map(
        _body, mesh=mesh,
        in_specs=(PartitionSpec("core"),) * nin,
        out_specs=(PartitionSpec("core"),) * len(out_names),
        check_rep=False))

    concat_in = [
        np.concatenate([np.asarray(in_maps[c][nm]) for c in range(N_CORES)],
                       axis=0)
        for nm in in_names
    ]
    concat_zeros = [
        np.zeros((N_CORES * z.shape[0], *z.shape[1:]), z.dtype)
        for z in zero_outs
    ]
    sharding = jax.sharding.NamedSharding(mesh, PartitionSpec("core"))
    dev_args = [jax.device_put(a, sharding) for a in concat_in + concat_zeros]

    out = sharded(*dev_args)
    jax.block_until_ready(out)
    t0 = time.perf_counter()
    for _ in range(n_iters):
        out = sharded(*dev_args)
    jax.block_until_ready(out)
    t1 = time.perf_counter()
    per_iter_ns = (t1 - t0) / n_iters * 1e9
    result = np.asarray(out[0]).reshape(N_CORES * NB, L, C)
    return per_iter_ns, result


# revision 12
# speedup vs baseline: 1.7059x; 1.0347x over previous
"""CBAM kernel for Trainium2, 8-way batch-parallel SPMD.

Computes out = x^2 * (att_c[b,c] + sigmoid(conv(spatial_stats))[b,l]) where
att_c = sigmoid(mlp(mean_L x) + mlp(max_L x)), matching the CBAM reference.

Layout per core: 4 batches; each batch x[4096, 256] lives in SBUF as one
[128, 8192] bf16 tensor (partition p holds DRAM rows [32p, 32p+32), i.e.
32 KiB contiguous fp32 DRAM per partition; the fp32->bf16 cast happens for
free inside the SWDGE load DMA). l = 32*p + j, free col = 256*j + c.

Engine split per batch (~23 us DMA window):
  DVE  : channel-max + spatial-max bf16 fold trees (2x perf mode), 8-j
         spatial-sum tree, the 32 final (att+sig)*x^2 STTs (deferred one
         batch so they fill the next load window), small reduces
  ACT  : 24-j spatial-sum accumulator copies, squares (bf16), sigmoids,
         PSUM->SBUF copies
  PE   : channel-sum (16x [128,512] bf16 matmuls), stats transposes, MLP,
         7-tap conv as banded-Toeplitz matmuls in transposed [j, p] space
         (corner taps = column-shifted rhs, no halo exchange needed)
  POOL : SWDGE DMA queue only - cast loads, cast stores (bf16 -> fp32)
"""

import numpy as np
from contextlib import ExitStack

import concourse.bacc as bacc
import concourse.bass as bass
import concourse.tile as tile
import concourse.mybir as mybir
from concourse.bass_utils import run_bass_kernel_spmd

AF = mybir.ActivationFunctionType
ALU = mybir.AluOpType
AX = mybir.AxisListType
FP32 = mybir.dt.float32
BF16 = mybir.dt.bfloat16

N_CORES = 8
B_FULL = 32
NB = B_FULL // N_CORES  # batches per core = 4
L = 4096
C = 256
HID = 16
HB = HID + 1
P = 128
NJ = L // P  # 32 j-blocks (rows per partition)
HJ = NJ // 2  # 16 j-blocks per half
HALF = HJ * C  # 4096 free columns per half

_CACHE: dict = {}


def _build_body(ctx: ExitStack, tc, out_d, x_d, w1_d, b1_d, w2b_d, id_d,
                id16_d, ones_d, rc16_d, tj_d, reps=1):
    nc = tc.nc

    const = ctx.enter_context(tc.tile_pool(name="const", bufs=1))
    bpool = ctx.enter_context(tc.tile_pool(name="xb16", bufs=3))
    opool = ctx.enter_context(tc.tile_pool(name="outt", bufs=2))
    sqpool = ctx.enter_context(tc.tile_pool(name="sq", bufs=8))
    mpool = ctx.enter_context(tc.tile_pool(name="maxtree", bufs=1))
    smpool = ctx.enter_context(tc.tile_pool(name="spattree", bufs=1))
    spool = ctx.enter_context(tc.tile_pool(name="stats", bufs=2))
    dpool = ctx.enter_context(tc.tile_pool(name="dummy", bufs=2))
    pacc = ctx.enter_context(tc.tile_pool(name="pacc", bufs=2, space="PSUM"))
    pwk = ctx.enter_context(tc.tile_pool(name="pwk", bufs=2, space="PSUM"))
    pcnv = ctx.enter_context(tc.tile_pool(name="pcnv", bufs=2, space="PSUM"))
    pb16 = ctx.enter_context(tc.tile_pool(name="pb16", bufs=2, space="PSUM"))

    w1 = const.tile([P, 2 * HB], FP32)
    nc.sync.dma_start(w1[:], w1_d[:])
    b1 = const.tile([HB, 1], FP32)
    nc.sync.dma_start(b1[:], b1_d[:])
    w2b = const.tile([HB, C], FP32)
    nc.sync.dma_start(w2b[:], w2b_d[:])
    ident = const.tile([P, P], FP32)
    nc.sync.dma_start(ident[:], id_d[:])
    ident16 = const.tile([P, P], BF16)
    nc.sync.dma_start(ident16[:], id16_d[:])
    ones = const.tile([P, P], FP32)
    nc.sync.dma_start(ones[:], ones_d[:])
    redcol16 = const.tile([P, 1], BF16)
    nc.sync.dma_start(redcol16[:], rc16_d[:])
    tj = const.tile([NJ, 6 * NJ], FP32)
    nc.sync.dma_start(tj[:], tj_d[:])

    NSA = 20  # j-blocks whose spatial sum rides ACT accumulator copies

    def emit_final(prev):
        """Final combine (att + sig) * x^2 for the previous batch + stores.

        Emitted at the top of the next iteration so the 32 DVE STTs fill
        the load window of the current batch, and the stores land on the
        SWDGE queue right behind the current batch's loads."""
        att, psig, sqs, pb = prev
        ot = opool.tile([P, NJ * C], FP32, tag="ot", name="ot")
        for s in range(4):
            for j8 in range(8):
                j = 8 * s + j8
                nc.vector.scalar_tensor_tensor(ot[:, C * j:C * (j + 1)],
                                               att[:], psig[:, j:j + 1],
                                               sqs[s][:, C * j8:C * (j8 + 1)],
                                               op0=ALU.add, op1=ALU.mult)
        ov = out_d[pb, :, :].rearrange("(p q) c -> p (q c)", p=P)
        nc.sync.dma_start(ov[:, 0:HALF], ot[:, 0:HALF])
        nc.sync.dma_start(ov[:, HALF:2 * HALF], ot[:, HALF:2 * HALF])

    prev = None
    seq = [b for _ in range(reps) for b in range(NB)]
    for it, b in enumerate(seq):
        last = it == len(seq) - 1
        xb = bpool.tile([P, NJ * C], BF16, tag="xb", name="xb")
        xv = x_d[b, :, :].rearrange("(p q) c -> p (q c)", p=P)
        QW = HALF // 2
        for q4 in range(4):
            nc.gpsimd.dma_start(xb[:, QW * q4:QW * (q4 + 1)],
                                xv[:, QW * q4:QW * (q4 + 1)])

        # On the last iteration the current batch's stats chain is the
        # critical path to the final stores - emit it before the previous
        # batch's (already-gated) combine so DVE prioritises it.
        if prev is not None and not last:
            emit_final(prev)
        lastprev = prev if last else None

        sej = spool.tile([P, NJ], FP32, tag="sej", name="sej")
        semf = spool.tile([P, NJ], FP32, tag="semf", name="semf")
        pcs = pacc.tile([1, 2 * C], FP32, tag="pcs")
        sqs = []
        for h in range(2):
            # ---- ACT: spatial-sum accumulator copies (j < NSA) ----
            for jh in range(HJ):
                j = HJ * h + jh
                if j < NSA:
                    dummy = dpool.tile([P, C], BF16, tag="dummy")
                    nc.scalar.activation(dummy[:], xb[:, C * j:C * (j + 1)],
                                         AF.Identity,
                                         accum_out=sej[:, j:j + 1])
            # ---- ACT: squares (feed next iteration's final combine) ----
            for q in range(2):
                sq = sqpool.tile([P, 8 * C], BF16, tag="sq")
                off = (2 * h + q) * 8 * C
                nc.scalar.activation(sq[:], xb[:, off:off + 8 * C], AF.Square)
                sqs.append(sq)
            # ---- PE: channel sum, 8x [128, 512] bf16 matmuls per half ----
            for m8 in range(8):
                m = 8 * h + m8
                nc.tensor.matmul(pcs[:], redcol16[:],
                                 xb[:, 512 * m:512 * (m + 1)],
                                 start=(m == 0), stop=(m == 15),
                                 skip_group_check=True)

        # ---- DVE: channel max over j, flat bf16 fold tree ----
        mh = mpool.tile([P, NJ * C // 2], BF16, tag="mh", name="mh")
        nc.vector.tensor_max(mh[:], xb[:, 0:HALF], xb[:, HALF:2 * HALF])
        w = HALF // 2
        while w >= C:
            nc.vector.tensor_max(mh[:, 0:w], mh[:, 0:w], mh[:, w:2 * w])
            w //= 2

        # ---- DVE: spatial max over c, strided bf16 fold tree ----
        sm = smpool.tile([P, NJ * (C // 2)], BF16, tag="sm", name="sm")
        sm3 = sm[:].rearrange("p (j c) -> p j c", c=C // 2)
        v3 = xb[:].rearrange("p (j c) -> p j c", c=C)
        nc.vector.tensor_max(sm3[:, :, :], v3[:, :, 0:C // 2],
                             v3[:, :, C // 2:C])
        w = C // 4
        while w >= 1:
            out = (semf[:, :].rearrange("p (j o) -> p j o", o=1)
                   if w == 1 else sm3[:, :, 0:w])
            nc.vector.tensor_max(out, sm3[:, :, 0:w], sm3[:, :, w:2 * w])
            w //= 2

        # ---- DVE: spatial sum for j >= NSA, small bf16 add tree ----
        nrem = NJ - NSA
        ss = smpool.tile([P, nrem * (C // 2)], BF16, tag="ss", name="ss")
        ss3 = ss[:].rearrange("p (j c) -> p j c", c=C // 2)
        v8 = xb[:, C * NSA:C * NJ].rearrange("p (j c) -> p j c", c=C)
        nc.vector.tensor_add(ss3[:, :, :], v8[:, :, 0:C // 2],
                             v8[:, :, C // 2:C])
        w = C // 4
        while w >= 1:
            out = (sej[:, NSA:NJ].rearrange("p (j o) -> p j o", o=1)
                   if w == 1 else ss3[:, :, 0:w])
            nc.vector.tensor_add(out, ss3[:, :, 0:w], ss3[:, :, w:2 * w])
            w //= 2

        # ---- channel stats into c-major [128, 4] via PE transposes ----
        avgw = spool.tile([1, 2 * C], FP32, tag="avgw", name="avgw")
        nc.scalar.copy(avgw[:], pcs[0:1, :])
        avg_row = spool.tile([1, C], FP32, tag="avg", name="avg")
        nc.vector.tensor_add(avg_row[:], avgw[0:1, 0:C], avgw[0:1, C:2 * C])
        stats = spool.tile([P, 4], FP32, tag="stats", name="stats")
        # fp32 PSUM bank: po 0:256, ph 256:258, pT 258:260
        wk = pwk.tile([P, C + 4], FP32, tag="wk")
        pT = wk[:, C + 2:C + 4]
        nc.tensor.transpose(pT[:, 0:1], avg_row[0:1, 0:P], ident[0:1, 0:1])
        nc.tensor.transpose(pT[:, 1:2], avg_row[0:1, P:C], ident[0:1, 0:1])
        # bf16 PSUM bank: mT 0:256, sig 256:288
        mtp = pb16.tile([P, 2 * P + NJ], BF16, tag="mtp")
        mT = mtp[:, 0:2 * P]
        nc.tensor.transpose(mT[:, 0:P], mh[:, 0:P], ident16[:, :])
        nc.tensor.transpose(mT[:, P:2 * P], mh[:, P:C], ident16[:, :])
        nc.scalar.copy(stats[:, 0:1], pT[:, 0:1])
        nc.scalar.copy(stats[:, 2:3], pT[:, 1:2])
        nc.vector.tensor_reduce(stats[:, 1:2], mT[:, 0:P],
                                axis=AX.X, op=ALU.max)
        nc.vector.tensor_reduce(stats[:, 3:4], mT[:, P:2 * P],
                                axis=AX.X, op=ALU.max)

        # ---- shared MLP: row HID carries the 2*b2 constant trick ----
        ph = wk[0:HB, C:C + 2]
        nc.tensor.matmul(ph[:], w1[:, 0:HB], stats[:, 0:2],
                         start=True, stop=False, skip_group_check=True)
        nc.tensor.matmul(ph[:], w1[:, HB:2 * HB], stats[:, 2:4],
                         start=False, stop=True, skip_group_check=True)
        hsb = spool.tile([HB, 2], FP32, tag="hsb", name="hsb")
        nc.scalar.activation(hsb[:], ph[:], AF.Relu, bias=b1[:])
        h2 = spool.tile([HB, 1], FP32, tag="h2", name="h2")
        nc.vector.tensor_add(h2[:], hsb[:, 0:1], hsb[:, 1:2])
        h2r = spool.tile([HB, P], FP32, tag="h2r", name="h2r")
        nc.scalar.mul(h2r[:], ones[0:HB, :], h2[:])
        po = wk[:, 0:C]
        nc.tensor.matmul(po[:], h2r[:], w2b[:], start=True, stop=True,
                         skip_group_check=True)
        att = spool.tile([P, C], BF16, tag="att", name="att")
        nc.scalar.activation(att[:], po[:], AF.Sigmoid)

        # ---- 7-tap conv in transposed [j, p] space (PE Toeplitz) ----
        # fp32 PSUM bank: sjT 0:128, smT 128:256, pcv 256:384
        pct = pcnv.tile([NJ, 3 * P], FP32, tag="pct")
        nc.tensor.transpose(pct[:, 0:P], sej[:], ident[:, :])
        nc.tensor.transpose(pct[:, P:2 * P], semf[:], ident[:, :])
        sjS = spool.tile([NJ, P], FP32, tag="sjS", name="sjS")
        nc.scalar.copy(sjS[:], pct[:, 0:P])
        smS = spool.tile([NJ, P], FP32, tag="smS", name="smS")
        nc.scalar.copy(smS[:], pct[:, P:2 * P])
        pcv = pct[:, 2 * P:3 * P]
        nc.tensor.matmul(pcv[:, :], tj[:, 0:NJ], sjS[:, :],
                         start=True, stop=False, skip_group_check=True)
        nc.tensor.matmul(pcv[:, 1:P], tj[:, NJ:2 * NJ], sjS[:, 0:P - 1],
                         start=False, stop=False, skip_group_check=True)
        nc.tensor.matmul(pcv[:, 0:P - 1], tj[:, 2 * NJ:3 * NJ], sjS[:, 1:P],
                         start=False, stop=False, skip_group_check=True)
        nc.tensor.matmul(pcv[:, :], tj[:, 3 * NJ:4 * NJ], smS[:, :],
                         start=False, stop=False, skip_group_check=True)
        nc.tensor.matmul(pcv[:, 1:P], tj[:, 4 * NJ:5 * NJ], smS[:, 0:P - 1],
                         start=False, stop=False, skip_group_check=True)
        nc.tensor.matmul(pcv[:, 0:P - 1], tj[:, 5 * NJ:6 * NJ], smS[:, 1:P],
                         start=False, stop=True, skip_group_check=True)
        sigT = spool.tile([NJ, P], BF16, tag="sigT", name="sigT")
        nc.scalar.activation(sigT[:], pcv[:], AF.Sigmoid)
        psig = mtp[:, 2 * P:2 * P + NJ]
        nc.tensor.transpose(psig[:], sigT[:], ident16[0:NJ, 0:NJ])
        sig_sb = spool.tile([P, NJ], BF16, tag="sig_sb", name="sig_sb")
        nc.scalar.copy(sig_sb[:], psig[:])

        if lastprev is not None:
            emit_final(lastprev)
        prev = (att, sig_sb, sqs, b)

    emit_final(prev)


def _build_nc(reps=1):
    nc = bacc.Bacc("TRN2", target_bir_lowering=False, debug=False,
                   enable_asserts=False, num_devices=N_CORES)
    x_d = nc.dram_tensor("xin", [NB, L, C], FP32, kind="ExternalInput").ap()
    w1_d = nc.dram_tensor("w1sb", [P, 2 * HB], FP32, kind="ExternalInput").ap()
    b1_d = nc.dram_tensor("b1col", [HB, 1], FP32, kind="ExternalInput").ap()
    w2b_d = nc.dram_tensor("w2b", [HB, C], FP32, kind="ExternalInput").ap()
    id_d = nc.dram_tensor("ident", [P, P], FP32, kind="ExternalInput").ap()
    id16_d = nc.dram_tensor("ident16", [P, P], BF16, kind="ExternalInput").ap()
    ones_d = nc.dram_tensor("ones", [P, P], FP32, kind="ExternalInput").ap()
    rc16_d = nc.dram_tensor("redcol16", [P, 1], BF16, kind="ExternalInput").ap()
    tj_d = nc.dram_tensor("tjconv", [NJ, 6 * NJ], FP32, kind="ExternalInput").ap()
    out_d = nc.dram_tensor("out", [NB, L, C], FP32, kind="ExternalOutput").ap()

    with tile.TileContext(nc) as tc:
        with ExitStack() as ctx:
            _build_body(ctx, tc, out_d, x_d, w1_d, b1_d, w2b_d, id_d,
                        id16_d, ones_d, rc16_d, tj_d, reps=reps)
    nc.compile()
    return nc


def get_nc(reps=1):
    key = f"nc{reps}"
    if key not in _CACHE:
        _CACHE[key] = _build_nc(reps=reps)
    return _CACHE[key]


def _prep_inputs(W1, b1, W2, b2, conv_w):
    """Host-side parameter preprocessing (shared across cores)."""
    W1 = np.asarray(W1, np.float32)
    W2 = np.asarray(W2, np.float32)
    b1 = np.asarray(b1, np.float32)
    b2 = np.asarray(b2, np.float32)
    conv_w = np.asarray(conv_w, np.float32)

    w1sb = np.zeros((P, 2 * HB), np.float32)
    for h in range(2):
        w1sb[:, HB * h:HB * h + HID] = W1[P * h:P * (h + 1), :]
    w2b = np.concatenate([W2, b2[None, :]], axis=0).astype(np.float32)
    b1col = np.concatenate([b1, [1.0]]).astype(np.float32).reshape(HB, 1)

    # Transposed-space conv Toeplitz lhsTs [j', j]; the avg tap folds in the
    # 1/C spatial-mean scale (device computes raw channel sums).
    wa = (conv_w[:, 0, 0] / C).astype(np.float32)
    wm = conv_w[:, 1, 0].astype(np.float32)
    tj = np.zeros((NJ, 6 * NJ), np.float32)
    for jp in range(NJ):
        for j in range(NJ):
            k = jp - j + 3          # main band
            if 0 <= k < 7:
                tj[jp, j] = wa[k]
                tj[jp, 3 * NJ + j] = wm[k]
            k = jp - j - 29         # prev-partition corner
            if 0 <= k < 7 and jp >= 29 and j <= 2:
                tj[jp, NJ + j] = wa[k]
                tj[jp, 4 * NJ + j] = wm[k]
            k = jp + 35 - j         # next-partition corner
            if 0 <= k < 7 and jp <= 2 and j >= 29:
                tj[jp, 2 * NJ + j] = wa[k]
                tj[jp, 5 * NJ + j] = wm[k]

    import ml_dtypes
    bf16 = ml_dtypes.bfloat16
    return {
        "w1sb": w1sb,
        "b1col": np.ascontiguousarray(b1col),
        "w2b": w2b,
        "ident": np.eye(P, dtype=np.float32),
        "ident16": np.eye(P, dtype=bf16),
        "ones": np.ones((P, P), np.float32),
        "redcol16": np.full((P, 1), 1.0 / L, bf16),
        "tjconv": tj,
    }


def kernel(x, W1, b1, W2, b2, conv_w):
    nc = get_nc()
    x = np.asarray(x, np.float32)
    params = _prep_inputs(W1, b1, W2, b2, conv_w)
    in_maps = []
    for c in range(N_CORES):
        m = dict(params)
        m["xin"] = np.ascontiguousarray(x[NB * c:NB * (c + 1)])
        in_maps.append(m)
    _CACHE["last_in_maps"] = in_maps
    res = run_bass_kernel_spmd(nc, in_maps, list(range(N_CORES)))
    _CACHE["last_results"] = res
    return np.concatenate([res.results[c]["out"] for c in range(N_CORES)],
                          axis=0)


def _pjrt_exec(nc, in_maps, n_warm=2, n_time=8):
    """Build a sharded jit for nc, run it, return (best_wall_s, result)."""
    import time
    import jax
    import concourse.mybir as mybir_
    from concourse.bass2jax import (_bass_exec_p, install_neuronx_cc_hook,
                                    partition_id_tensor)
    from jax.experimental.shard_map import shard_map
    from jax.sharding import Mesh, PartitionSpec

    install_neuronx_cc_hook()
    partition_name = (nc.partition_id_tensor.name
                      if nc.partition_id_tensor else None)
    in_names, out_names, out_avals = [], [], []
    for alloc in nc.m.functions[0].allocations:
        if not isinstance(alloc, mybir_.MemoryLocationSet):
            continue
        name = alloc.memorylocations[0].name
        if alloc.kind == "ExternalInput":
            if name != partition_name:
                in_names.append(name)
        elif alloc.kind == "ExternalOutput":
            out_names.append(name)
            out_avals.append(jax.core.ShapedArray(
                tuple(alloc.tensor_shape), mybir_.dt.np(alloc.dtype)))
    n_params = len(in_names)
    all_in_names = list(in_names) + list(out_names)
    if partition_name is not None:
        all_in_names.append(partition_name)

    def _body(*args):
        operands = list(args)
        if partition_name is not None:
            operands.append(partition_id_tensor())
        return tuple(_bass_exec_p.bind(
            *operands,
            out_avals=tuple(out_avals),
            in_names=tuple(all_in_names),
            out_names=tuple(out_names),
            lowering_input_output_aliases=(),
            sim_require_finite=True,
            sim_require_nnan=True,
            nc=nc,
        ))

    devices = jax.devices()[:N_CORES]
    mesh = Mesh(np.asarray(devices), ("core",))
    nin = n_params + len(out_names)
    sharding = jax.sharding.NamedSharding(mesh, PartitionSpec("core"))
    fn = jax.jit(shard_map(
        _body, mesh=mesh,
        in_specs=(PartitionSpec("core"),) * nin,
        out_specs=(PartitionSpec("core"),) * len(out_names),
        check_rep=False))
    dev_args = [
        jax.device_put(np.concatenate(
            [np.asarray(in_maps[c][nm]) for c in range(N_CORES)], axis=0),
            sharding)
        for nm in in_names
    ]
    for av in out_avals:
        z = np.zeros((N_CORES * av.shape[0], *av.shape[1:]), av.dtype)
        dev_args.append(jax.device_put(z, sharding))

    for _ in range(n_warm):
        out = fn(*dev_args)
        jax.block_until_ready(out)
    best = float("inf")
    for _ in range(n_time):
        t0 = time.perf_counter()
        out = fn(*dev_args)
        jax.block_until_ready(out)
        best = min(best, time.perf_counter() - t0)
    result = np.asarray(out[0]).reshape(N_CORES * NB, L, C)
    return best, result


def bench_repeat(reps=8, n_time=10, in_maps=None):
    """Isolate device exec time: time a module doing the work `reps` times
    in-kernel vs once; slope = steady-state HW time per execution."""
    if in_maps is None:
        in_maps = _CACHE["last_in_maps"]
    t1, _ = _pjrt_exec(get_nc(1), in_maps, n_time=n_time)
    tr, result = _pjrt_exec(get_nc(reps), in_maps, n_time=n_time)
    per_exec_ns = (tr - t1) / (reps - 1) * 1e9
    return per_exec_ns, result, t1 * 1e9, tr * 1e9


def bench(n_iters=30, in_maps=None):
    """Time back-to-back NEFF executions with device-resident inputs."""
    import time
    import jax
    import concourse.mybir as mybir_
    from concourse.bass2jax import (_bass_exec_p, install_neuronx_cc_hook,
                                    partition_id_tensor)
    from jax.experimental.shard_map import shard_map
    from jax.sharding import Mesh, PartitionSpec

    nc = get_nc()
    if in_maps is None:
        in_maps = _CACHE["last_in_maps"]
    install_neuronx_cc_hook()

    partition_name = (nc.partition_id_tensor.name
                      if nc.partition_id_tensor else None)
    in_names, out_names, out_avals, zero_outs = [], [], [], []
    for alloc in nc.m.functions[0].allocations:
        if not isinstance(alloc, mybir_.MemoryLocationSet):
            continue
        name = alloc.memorylocations[0].name
        if alloc.kind == "ExternalInput":
            if name != partition_name:
                in_names.append(name)
        elif alloc.kind == "ExternalOutput":
            shape = tuple(alloc.tensor_shape)
            dtype = mybir_.dt.np(alloc.dtype)
            out_names.append(name)
            out_avals.append(jax.core.ShapedArray(shape, dtype))
            zero_outs.append(np.zeros(shape, dtype))
    n_params = len(in_names)
    all_in_names = list(in_names) + list(out_names)
    if partition_name is not None:
        all_in_names.append(partition_name)

    def _body(*args):
        operands = list(args)
        if partition_name is not None:
            operands.append(partition_id_tensor())
        return tuple(_bass_exec_p.bind(
            *operands,
            out_avals=tuple(out_avals),
            in_names=tuple(all_in_names),
            out_names=tuple(out_names),
            lowering_input_output_aliases=(),
            sim_require_finite=True,
            sim_require_nnan=True,
            nc=nc,
        ))

    devices = jax.devices()[:N_CORES]
    mesh = Mesh(np.asarray(devices), ("core",))
    nin = n_params + len(out_names)
    sharded = jax.jit(shard_map(
        _body, mesh=mesh,
        in_specs=(PartitionSpec("core"),) * nin,
        out_specs=(PartitionSpec("core"),) * len(out_names),
        check_rep=False))

    concat_in = [
        np.concatenate([np.asarray(in_maps[c][nm]) for c in range(N_CORES)],
                       axis=0)
        for nm in in_names
    ]
    concat_zeros = [
        np.zeros((N_CORES * z.shape[0], *z.shape[1:]), z.dtype)
        for z in zero_outs
    ]
    sharding = jax.sharding.NamedSharding(mesh, PartitionSpec("core"))
    dev_args = [jax.device_put(a, sharding) for a in concat_in + concat_zeros]

    out = sharded(*dev_args)
    jax.block_until_ready(out)
    t0 = time.perf_counter()
    for _ in range(n_iters):
        out = sharded(*dev_args)
    jax.block_until_ready(out)
    t1 = time.perf_counter()
    per_iter_ns = (t1 - t0) / n_iters * 1e9
    result = np.asarray(out[0]).reshape(N_CORES * NB, L, C)
    return per_iter_ns, result


# revision 13
# speedup vs baseline: 1.7238x; 1.0105x over previous
"""CBAM kernel for Trainium2, 8-way batch-parallel SPMD.

Computes out = x^2 * (att_c[b,c] + sigmoid(conv(spatial_stats))[b,l]) where
att_c = sigmoid(mlp(mean_L x) + mlp(max_L x)), matching the CBAM reference.

Layout per core: 4 batches; each batch x[4096, 256] lives in SBUF as one
[128, 8192] bf16 tensor (partition p holds DRAM rows [32p, 32p+32), i.e.
32 KiB contiguous fp32 DRAM per partition; the fp32->bf16 cast happens for
free inside the SWDGE load DMA). l = 32*p + j, free col = 256*j + c.

Engine split per batch (~23 us DMA window):
  DVE  : channel-max + spatial-max bf16 fold trees (2x perf mode), 8-j
         spatial-sum tree, the 32 final (att+sig)*x^2 STTs (deferred one
         batch so they fill the next load window), small reduces
  ACT  : 24-j spatial-sum accumulator copies, squares (bf16), sigmoids,
         PSUM->SBUF copies
  PE   : channel-sum (16x [128,512] bf16 matmuls), stats transposes, MLP,
         7-tap conv as banded-Toeplitz matmuls in transposed [j, p] space
         (corner taps = column-shifted rhs, no halo exchange needed)
  POOL : SWDGE DMA queue only - cast loads, cast stores (bf16 -> fp32)
"""

import numpy as np
from contextlib import ExitStack

import concourse.bacc as bacc
import concourse.bass as bass
import concourse.tile as tile
import concourse.mybir as mybir
from concourse.bass_utils import run_bass_kernel_spmd

AF = mybir.ActivationFunctionType
ALU = mybir.AluOpType
AX = mybir.AxisListType
FP32 = mybir.dt.float32
BF16 = mybir.dt.bfloat16

N_CORES = 8
B_FULL = 32
NB = B_FULL // N_CORES  # batches per core = 4
L = 4096
C = 256
HID = 16
HB = HID + 1
P = 128
NJ = L // P  # 32 j-blocks (rows per partition)
HJ = NJ // 2  # 16 j-blocks per half
HALF = HJ * C  # 4096 free columns per half

_CACHE: dict = {}


def _build_body(ctx: ExitStack, tc, out_d, x_d, w1_d, b1_d, w2b_d, id_d,
                id16_d, ones_d, rc16_d, tj_d, reps=1):
    nc = tc.nc

    const = ctx.enter_context(tc.tile_pool(name="const", bufs=1))
    bpool = ctx.enter_context(tc.tile_pool(name="xb16", bufs=3))
    opool = ctx.enter_context(tc.tile_pool(name="outt", bufs=2))
    sqpool = ctx.enter_context(tc.tile_pool(name="sq", bufs=4))
    mpool = ctx.enter_context(tc.tile_pool(name="maxtree", bufs=1))
    smpool = ctx.enter_context(tc.tile_pool(name="spattree", bufs=1))
    spool = ctx.enter_context(tc.tile_pool(name="stats", bufs=2))
    dpool = ctx.enter_context(tc.tile_pool(name="dummy", bufs=2))
    pacc = ctx.enter_context(tc.tile_pool(name="pacc", bufs=2, space="PSUM"))
    pwk = ctx.enter_context(tc.tile_pool(name="pwk", bufs=2, space="PSUM"))
    pcnv = ctx.enter_context(tc.tile_pool(name="pcnv", bufs=2, space="PSUM"))
    pb16 = ctx.enter_context(tc.tile_pool(name="pb16", bufs=2, space="PSUM"))

    w1 = const.tile([P, 2 * HB], FP32)
    nc.sync.dma_start(w1[:], w1_d[:])
    b1 = const.tile([HB, 1], FP32)
    nc.sync.dma_start(b1[:], b1_d[:])
    w2b = const.tile([HB, C], FP32)
    nc.sync.dma_start(w2b[:], w2b_d[:])
    ident = const.tile([P, P], FP32)
    nc.sync.dma_start(ident[:], id_d[:])
    ident16 = const.tile([P, P], BF16)
    nc.sync.dma_start(ident16[:], id16_d[:])
    ones = const.tile([P, P], FP32)
    nc.sync.dma_start(ones[:], ones_d[:])
    redcol16 = const.tile([P, 1], BF16)
    nc.sync.dma_start(redcol16[:], rc16_d[:])
    tj = const.tile([NJ, 6 * NJ], FP32)
    nc.sync.dma_start(tj[:], tj_d[:])

    NSA = 20  # j-blocks whose spatial sum rides ACT accumulator copies

    def emit_final(prev):
        """Final combine (att + sig) * x^2 for the previous batch + stores.

        Emitted at the top of the next iteration so the 32 DVE STTs fill
        the load window of the current batch, and the stores land on the
        SWDGE queue right behind the current batch's loads."""
        att, psig, sqs, pb = prev
        ot = opool.tile([P, NJ * C], FP32, tag="ot", name="ot")
        for j in range(NJ):
            jh = j % HJ
            nc.vector.scalar_tensor_tensor(ot[:, C * j:C * (j + 1)],
                                           att[:], psig[:, j:j + 1],
                                           sqs[j // HJ][:, C * jh:C * (jh + 1)],
                                           op0=ALU.add, op1=ALU.mult)
        ov = out_d[pb, :, :].rearrange("(p q) c -> p (q c)", p=P)
        nc.sync.dma_start(ov[:, 0:HALF], ot[:, 0:HALF])
        nc.sync.dma_start(ov[:, HALF:2 * HALF], ot[:, HALF:2 * HALF])

    prev = None
    seq = [b for _ in range(reps) for b in range(NB)]
    for it, b in enumerate(seq):
        last = it == len(seq) - 1
        xb = bpool.tile([P, NJ * C], BF16, tag="xb", name="xb")
        xv = x_d[b, :, :].rearrange("(p q) c -> p (q c)", p=P)
        QW = HALF // 2
        for q4 in range(4):
            nc.gpsimd.dma_start(xb[:, QW * q4:QW * (q4 + 1)],
                                xv[:, QW * q4:QW * (q4 + 1)])

        # On the last iteration the current batch's stats chain is the
        # critical path to the final stores - emit it before the previous
        # batch's (already-gated) combine so DVE prioritises it.
        if prev is not None and not last:
            emit_final(prev)
        lastprev = prev if last else None

        sej = spool.tile([P, NJ], FP32, tag="sej", name="sej")
        semf = spool.tile([P, NJ], FP32, tag="semf", name="semf")
        pcs = pacc.tile([1, 2 * C], FP32, tag="pcs")
        sqs = []
        for h in range(2):
            # ---- ACT: spatial-sum accumulator copies (j < NSA) ----
            for jh in range(HJ):
                j = HJ * h + jh
                if j < NSA:
                    dummy = dpool.tile([P, C], BF16, tag="dummy")
                    nc.scalar.activation(dummy[:], xb[:, C * j:C * (j + 1)],
                                         AF.Identity,
                                         accum_out=sej[:, j:j + 1])
            # ---- PE: channel sum, 8x [128, 512] bf16 matmuls per half ----
            for m8 in range(8):
                m = 8 * h + m8
                nc.tensor.matmul(pcs[:], redcol16[:],
                                 xb[:, 512 * m:512 * (m + 1)],
                                 start=(m == 0), stop=(m == 15),
                                 skip_group_check=True)

        # ---- DVE: channel max over j, flat bf16 fold tree ----
        mh = mpool.tile([P, NJ * C // 2], BF16, tag="mh", name="mh")
        nc.vector.tensor_max(mh[:], xb[:, 0:HALF], xb[:, HALF:2 * HALF])
        w = HALF // 2
        while w >= C:
            nc.vector.tensor_max(mh[:, 0:w], mh[:, 0:w], mh[:, w:2 * w])
            w //= 2

        # ---- DVE: spatial max over c, strided bf16 fold tree ----
        sm = smpool.tile([P, NJ * (C // 2)], BF16, tag="sm", name="sm")
        sm3 = sm[:].rearrange("p (j c) -> p j c", c=C // 2)
        v3 = xb[:].rearrange("p (j c) -> p j c", c=C)
        nc.vector.tensor_max(sm3[:, :, :], v3[:, :, 0:C // 2],
                             v3[:, :, C // 2:C])
        w = C // 4
        while w >= 1:
            out = (semf[:, :].rearrange("p (j o) -> p j o", o=1)
                   if w == 1 else sm3[:, :, 0:w])
            nc.vector.tensor_max(out, sm3[:, :, 0:w], sm3[:, :, w:2 * w])
            w //= 2

        # ---- DVE: spatial sum for j >= NSA, small bf16 add tree ----
        nrem = NJ - NSA
        ss = smpool.tile([P, nrem * (C // 2)], BF16, tag="ss", name="ss")
        ss3 = ss[:].rearrange("p (j c) -> p j c", c=C // 2)
        v8 = xb[:, C * NSA:C * NJ].rearrange("p (j c) -> p j c", c=C)
        nc.vector.tensor_add(ss3[:, :, :], v8[:, :, 0:C // 2],
                             v8[:, :, C // 2:C])
        w = C // 4
        while w >= 1:
            out = (sej[:, NSA:NJ].rearrange("p (j o) -> p j o", o=1)
                   if w == 1 else ss3[:, :, 0:w])
            nc.vector.tensor_add(out, ss3[:, :, 0:w], ss3[:, :, w:2 * w])
            w //= 2

        # ---- channel stats into c-major [128, 4] via PE transposes ----
        avgw = spool.tile([1, 2 * C], FP32, tag="avgw", name="avgw")
        nc.scalar.copy(avgw[:], pcs[0:1, :])
        avg_row = spool.tile([1, C], FP32, tag="avg", name="avg")
        nc.vector.tensor_add(avg_row[:], avgw[0:1, 0:C], avgw[0:1, C:2 * C])
        stats = spool.tile([P, 4], FP32, tag="stats", name="stats")
        # fp32 PSUM bank: po 0:256, ph 256:258, pT 258:260
        wk = pwk.tile([P, C + 4], FP32, tag="wk")
        pT = wk[:, C + 2:C + 4]
        nc.tensor.transpose(pT[:, 0:1], avg_row[0:1, 0:P], ident[0:1, 0:1])
        nc.tensor.transpose(pT[:, 1:2], avg_row[0:1, P:C], ident[0:1, 0:1])
        # bf16 PSUM bank: mT 0:256, sig 256:288
        mtp = pb16.tile([P, 2 * P + NJ], BF16, tag="mtp")
        mT = mtp[:, 0:2 * P]
        nc.tensor.transpose(mT[:, 0:P], mh[:, 0:P], ident16[:, :])
        nc.tensor.transpose(mT[:, P:2 * P], mh[:, P:C], ident16[:, :])
        nc.scalar.copy(stats[:, 0:1], pT[:, 0:1])
        nc.scalar.copy(stats[:, 2:3], pT[:, 1:2])
        nc.vector.tensor_reduce(stats[:, 1:2], mT[:, 0:P],
                                axis=AX.X, op=ALU.max)
        nc.vector.tensor_reduce(stats[:, 3:4], mT[:, P:2 * P],
                                axis=AX.X, op=ALU.max)

        # ---- shared MLP: row HID carries the 2*b2 constant trick ----
        ph = wk[0:HB, C:C + 2]
        nc.tensor.matmul(ph[:], w1[:, 0:HB], stats[:, 0:2],
                         start=True, stop=False, skip_group_check=True)
        nc.tensor.matmul(ph[:], w1[:, HB:2 * HB], stats[:, 2:4],
                         start=False, stop=True, skip_group_check=True)
        hsb = spool.tile([HB, 2], FP32, tag="hsb", name="hsb")
        nc.scalar.activation(hsb[:], ph[:], AF.Relu, bias=b1[:])
        h2 = spool.tile([HB, 1], FP32, tag="h2", name="h2")
        nc.vector.tensor_add(h2[:], hsb[:, 0:1], hsb[:, 1:2])
        h2r = spool.tile([HB, P], FP32, tag="h2r", name="h2r")
        nc.scalar.mul(h2r[:], ones[0:HB, :], h2[:])
        po = wk[:, 0:C]
        nc.tensor.matmul(po[:], h2r[:], w2b[:], start=True, stop=True,
                         skip_group_check=True)
        att = spool.tile([P, C], BF16, tag="att", name="att")
        nc.scalar.activation(att[:], po[:], AF.Sigmoid)

        # ---- 7-tap conv in transposed [j, p] space (PE Toeplitz) ----
        # fp32 PSUM bank: sjT 0:128, smT 128:256, pcv 256:384
        pct = pcnv.tile([NJ, 3 * P], FP32, tag="pct")
        nc.tensor.transpose(pct[:, 0:P], sej[:], ident[:, :])
        nc.tensor.transpose(pct[:, P:2 * P], semf[:], ident[:, :])
        sjS = spool.tile([NJ, P], FP32, tag="sjS", name="sjS")
        nc.scalar.copy(sjS[:], pct[:, 0:P])
        smS = spool.tile([NJ, P], FP32, tag="smS", name="smS")
        nc.scalar.copy(smS[:], pct[:, P:2 * P])
        pcv = pct[:, 2 * P:3 * P]
        nc.tensor.matmul(pcv[:, :], tj[:, 0:NJ], sjS[:, :],
                         start=True, stop=False, skip_group_check=True)
        nc.tensor.matmul(pcv[:, 1:P], tj[:, NJ:2 * NJ], sjS[:, 0:P - 1],
                         start=False, stop=False, skip_group_check=True)
        nc.tensor.matmul(pcv[:, 0:P - 1], tj[:, 2 * NJ:3 * NJ], sjS[:, 1:P],
                         start=False, stop=False, skip_group_check=True)
        nc.tensor.matmul(pcv[:, :], tj[:, 3 * NJ:4 * NJ], smS[:, :],
                         start=False, stop=False, skip_group_check=True)
        nc.tensor.matmul(pcv[:, 1:P], tj[:, 4 * NJ:5 * NJ], smS[:, 0:P - 1],
                         start=False, stop=False, skip_group_check=True)
        nc.tensor.matmul(pcv[:, 0:P - 1], tj[:, 5 * NJ:6 * NJ], smS[:, 1:P],
                         start=False, stop=True, skip_group_check=True)
        sigT = spool.tile([NJ, P], BF16, tag="sigT", name="sigT")
        nc.scalar.activation(sigT[:], pcv[:], AF.Sigmoid)
        psig = mtp[:, 2 * P:2 * P + NJ]
        nc.tensor.transpose(psig[:], sigT[:], ident16[0:NJ, 0:NJ])
        sig_sb = spool.tile([P, NJ], BF16, tag="sig_sb", name="sig_sb")
        nc.scalar.copy(sig_sb[:], psig[:])

        # ---- ACT: squares late (feed the NEXT iteration's combine) ----
        for hq in range(2):
            sq = sqpool.tile([P, NJ * C // 2], BF16, tag="sq")
            nc.scalar.activation(sq[:], xb[:, HALF * hq:HALF * (hq + 1)],
                                 AF.Square)
            sqs.append(sq)

        if lastprev is not None:
            emit_final(lastprev)
        prev = (att, sig_sb, sqs, b)

    emit_final(prev)


def _build_nc(reps=1):
    nc = bacc.Bacc("TRN2", target_bir_lowering=False, debug=False,
                   enable_asserts=False, num_devices=N_CORES)
    x_d = nc.dram_tensor("xin", [NB, L, C], FP32, kind="ExternalInput").ap()
    w1_d = nc.dram_tensor("w1sb", [P, 2 * HB], FP32, kind="ExternalInput").ap()
    b1_d = nc.dram_tensor("b1col", [HB, 1], FP32, kind="ExternalInput").ap()
    w2b_d = nc.dram_tensor("w2b", [HB, C], FP32, kind="ExternalInput").ap()
    id_d = nc.dram_tensor("ident", [P, P], FP32, kind="ExternalInput").ap()
    id16_d = nc.dram_tensor("ident16", [P, P], BF16, kind="ExternalInput").ap()
    ones_d = nc.dram_tensor("ones", [P, P], FP32, kind="ExternalInput").ap()
    rc16_d = nc.dram_tensor("redcol16", [P, 1], BF16, kind="ExternalInput").ap()
    tj_d = nc.dram_tensor("tjconv", [NJ, 6 * NJ], FP32, kind="ExternalInput").ap()
    out_d = nc.dram_tensor("out", [NB, L, C], FP32, kind="ExternalOutput").ap()

    with tile.TileContext(nc) as tc:
        with ExitStack() as ctx:
            _build_body(ctx, tc, out_d, x_d, w1_d, b1_d, w2b_d, id_d,
                        id16_d, ones_d, rc16_d, tj_d, reps=reps)
    nc.compile()
    return nc


def get_nc(reps=1):
    key = f"nc{reps}"
    if key not in _CACHE:
        _CACHE[key] = _build_nc(reps=reps)
    return _CACHE[key]


def _prep_inputs(W1, b1, W2, b2, conv_w):
    """Host-side parameter preprocessing (shared across cores)."""
    W1 = np.asarray(W1, np.float32)
    W2 = np.asarray(W2, np.float32)
    b1 = np.asarray(b1, np.float32)
    b2 = np.asarray(b2, np.float32)
    conv_w = np.asarray(conv_w, np.float32)

    w1sb = np.zeros((P, 2 * HB), np.float32)
    for h in range(2):
        w1sb[:, HB * h:HB * h + HID] = W1[P * h:P * (h + 1), :]
    w2b = np.concatenate([W2, b2[None, :]], axis=0).astype(np.float32)
    b1col = np.concatenate([b1, [1.0]]).astype(np.float32).reshape(HB, 1)

    # Transposed-space conv Toeplitz lhsTs [j', j]; the avg tap folds in the
    # 1/C spatial-mean scale (device computes raw channel sums).
    wa = (conv_w[:, 0, 0] / C).astype(np.float32)
    wm = conv_w[:, 1, 0].astype(np.float32)
    tj = np.zeros((NJ, 6 * NJ), np.float32)
    for jp in range(NJ):
        for j in range(NJ):
            k = jp - j + 3          # main band
            if 0 <= k < 7:
                tj[jp, j] = wa[k]
                tj[jp, 3 * NJ + j] = wm[k]
            k = jp - j - 29         # prev-partition corner
            if 0 <= k < 7 and jp >= 29 and j <= 2:
                tj[jp, NJ + j] = wa[k]
                tj[jp, 4 * NJ + j] = wm[k]
            k = jp + 35 - j         # next-partition corner
            if 0 <= k < 7 and jp <= 2 and j >= 29:
                tj[jp, 2 * NJ + j] = wa[k]
                tj[jp, 5 * NJ + j] = wm[k]

    import ml_dtypes
    bf16 = ml_dtypes.bfloat16
    return {
        "w1sb": w1sb,
        "b1col": np.ascontiguousarray(b1col),
        "w2b": w2b,
        "ident": np.eye(P, dtype=np.float32),
        "ident16": np.eye(P, dtype=bf16),
        "ones": np.ones((P, P), np.float32),
        "redcol16": np.full((P, 1), 1.0 / L, bf16),
        "tjconv": tj,
    }


def kernel(x, W1, b1, W2, b2, conv_w):
    nc = get_nc()
    x = np.asarray(x, np.float32)
    params = _prep_inputs(W1, b1, W2, b2, conv_w)
    in_maps = []
    for c in range(N_CORES):
        m = dict(params)
        m["xin"] = np.ascontiguousarray(x[NB * c:NB * (c + 1)])
        in_maps.append(m)
    _CACHE["last_in_maps"] = in_maps
    res = run_bass_kernel_spmd(nc, in_maps, list(range(N_CORES)))
    _CACHE["last_results"] = res
    return np.concatenate([res.results[c]["out"] for c in range(N_CORES)],
                          axis=0)


def _pjrt_exec(nc, in_maps, n_warm=2, n_time=8):
    """Build a sharded jit for nc, run it, return (best_wall_s, result)."""
    import time
    import jax
    import concourse.mybir as mybir_
    from concourse.bass2jax import (_bass_exec_p, install_neuronx_cc_hook,
                                    partition_id_tensor)
    from jax.experimental.shard_map import shard_map
    from jax.sharding import Mesh, PartitionSpec

    install_neuronx_cc_hook()
    partition_name = (nc.partition_id_tensor.name
                      if nc.partition_id_tensor else None)
    in_names, out_names, out_avals = [], [], []
    for alloc in nc.m.functions[0].allocations:
        if not isinstance(alloc, mybir_.MemoryLocationSet):
            continue
        name = alloc.memorylocations[0].name
        if alloc.kind == "ExternalInput":
            if name != partition_name:
                in_names.append(name)
        elif alloc.kind == "ExternalOutput":
            out_names.append(name)
            out_avals.append(jax.core.ShapedArray(
                tuple(alloc.tensor_shape), mybir_.dt.np(alloc.dtype)))
    n_params = len(in_names)
    all_in_names = list(in_names) + list(out_names)
    if partition_name is not None:
        all_in_names.append(partition_name)

    def _body(*args):
        operands = list(args)
        if partition_name is not None:
            operands.append(partition_id_tensor())
        return tuple(_bass_exec_p.bind(
            *operands,
            out_avals=tuple(out_avals),
            in_names=tuple(all_in_names),
            out_names=tuple(out_names),
            lowering_input_output_aliases=(),
            sim_require_finite=True,
            sim_require_nnan=True,
            nc=nc,
        ))

    devices = jax.devices()[:N_CORES]
    mesh = Mesh(np.asarray(devices), ("core",))
    nin = n_params + len(out_names)
    sharding = jax.sharding.NamedSharding(mesh, PartitionSpec("core"))
    fn = jax.jit(shard_map(
        _body, mesh=mesh,
        in_specs=(PartitionSpec("core"),) * nin,
        out_specs=(PartitionSpec("core"),) * len(out_names),
        check_rep=False))
    dev_args = [
        jax.device_put(np.concatenate(
            [np.asarray(in_maps[c][nm]) for c in range(N_CORES)], axis=0),
            sharding)
        for nm in in_names
    ]
    for av in out_avals:
        z = np.zeros((N_CORES * av.shape[0], *av.shape[1:]), av.dtype)
        dev_args.append(jax.device_put(z, sharding))

    for _ in range(n_warm):
        out = fn(*dev_args)
        jax.block_until_ready(out)
    best = float("inf")
    for _ in range(n_time):
        t0 = time.perf_counter()
        out = fn(*dev_args)
        jax.block_until_ready(out)
        best = min(best, time.perf_counter() - t0)
    result = np.asarray(out[0]).reshape(N_CORES * NB, L, C)
    return best, result


def bench_repeat(reps=8, n_time=10, in_maps=None):
    """Isolate device exec time: time a module doing the work `reps` times
    in-kernel vs once; slope = steady-state HW time per execution."""
    if in_maps is None:
        in_maps = _CACHE["last_in_maps"]
    t1, _ = _pjrt_exec(get_nc(1), in_maps, n_time=n_time)
    tr, result = _pjrt_exec(get_nc(reps), in_maps, n_time=n_time)
    per_exec_ns = (tr - t1) / (reps - 1) * 1e9
    return per_exec_ns, result, t1 * 1e9, tr * 1e9


def bench(n_iters=30, in_maps=None):
    """Time back-to-back NEFF executions with device-resident inputs."""
    import time
    import jax
    import concourse.mybir as mybir_
    from concourse.bass2jax import (_bass_exec_p, install_neuronx_cc_hook,
                                    partition_id_tensor)
    from jax.experimental.shard_map import shard_map
    from jax.sharding import Mesh, PartitionSpec

    nc = get_nc()
    if in_maps is None:
        in_maps = _CACHE["last_in_maps"]
    install_neuronx_cc_hook()

    partition_name = (nc.partition_id_tensor.name
                      if nc.partition_id_tensor else None)
    in_names, out_names, out_avals, zero_outs = [], [], [], []
    for alloc in nc.m.functions[0].allocations:
        if not isinstance(alloc, mybir_.MemoryLocationSet):
            continue
        name = alloc.memorylocations[0].name
        if alloc.kind == "ExternalInput":
            if name != partition_name:
                in_names.append(name)
        elif alloc.kind == "ExternalOutput":
            shape = tuple(alloc.tensor_shape)
            dtype = mybir_.dt.np(alloc.dtype)
            out_names.append(name)
            out_avals.append(jax.core.ShapedArray(shape, dtype))
            zero_outs.append(np.zeros(shape, dtype))
    n_params = len(in_names)
    all_in_names = list(in_names) + list(out_names)
    if partition_name is not None:
        all_in_names.append(partition_name)

    def _body(*args):
        operands = list(args)
        if partition_name is not None:
            operands.append(partition_id_tensor())
        return tuple(_bass_exec_p.bind(
            *operands,
            out_avals=tuple(out_avals),
            in_names=tuple(all_in_names),
            out_names=tuple(out_names),
            lowering_input_output_aliases=(),
            sim_require_finite=True,
            sim_require_nnan=True,
            nc=nc,
        ))

    devices = jax.devices()[:N_CORES]
    mesh = Mesh(np.asarray(devices), ("core",))
    nin = n_params + len(out_names)
    sharded = jax.jit(shard_map(
        _body, mesh=mesh,
        in_specs=(PartitionSpec("core"),) * nin,
        out_specs=(PartitionSpec("core"),) * len(out_names),
        check_rep=False))

    concat_in = [
        np.concatenate([np.asarray(in_maps[c][nm]) for c in range(N_CORES)],
                       axis=0)
        for nm in in_names
    ]
    concat_zeros = [
        np.zeros((N_CORES * z.shape[0], *z.shape[1:]), z.dtype)
        for z in zero_outs
    ]
    sharding = jax.sharding.NamedSharding(mesh, PartitionSpec("core"))
    dev_args = [jax.device_put(a, sharding) for a in concat_in + concat_zeros]

    out = sharded(*dev_args)
    jax.block_until_ready(out)
    t0 = time.perf_counter()
    for _ in range(n_iters):
        out = sharded(*dev_args)
    jax.block_until_ready(out)
    t1 = time.perf_counter()
    per_iter_ns = (t1 - t0) / n_iters * 1e9
    result = np.asarray(out[0]).reshape(N_CORES * NB, L, C)
    return per_iter_ns, result
